# revision 37
# baseline (speedup 1.0000x reference)
"""DecoderLSTM (attention + LSTM + vocab projection) on 8 Trainium2 NeuronCores.

Strategy (data-parallel over batch, no collectives):
  - Each of the 8 cores owns 4 of the 32 batch elements and runs the full
    64-step attention-LSTM recurrence for them in bf16 (fp32 cell state),
    storing h_t transposed in SBUF.
  - The vocab projection (90% of FLOPs) is hoisted out of the recurrence:
    one dense [256,512]@[512,32000] matmul per core, streaming out_W.T
    from HBM, partially interleaved into the recurrence's idle PE windows.
  - Algebraic folds done on host (numpy): the embedding gather, h0/c0 init,
    reshape_W folded into the LSTM input weights (W_cg = Wih @ R1), and the
    per-step embedding contribution G_emb[t] = emb_t @ (Wih R2).T + biases.

Wall-clock engineering (the axon tunnel runs at ~40 MB/s, so bytes moved
per call dominate end-to-end latency):
  - All weight tensors (wcgT/whhT/outWT/outb/gwT/gbias + identities) are
    baked into the NEFF as Const tensors (inline_tensor): they ride in the
    executable at load time and are NOT re-transferred per call. kernel()
    re-specializes (retrace + recompile) if the weight values change.
  - The PJRT executable is jitted ONCE per weight-set and cached; calls
    after the first skip trace/lower/compile entirely.
  - No zero output buffers are shipped (the kernel writes every output
    element, so uninitialized XLA output buffers are fine).
  - Per-call H2D is only the data-dependent inputs (~8.7 MB total):
    unpadded features (fP0/fP1), transposed token embeddings (embT), and
    h0/c0. fT (the transposed features layout) and G_emb (the gate-space
    embedding contribution) are derived on device.
  - The logits cross the tunnel int8-quantized with per-row scales
    (absmax/127, hardware round-to-nearest), ~66 MB instead of 262 MB
    f32; the host dequantizes. Output shards are fetched concurrently
    (jax.device_get on the shard list) — a lone np.asarray walks shards
    serially at ~2/3 the rate.

Numerics: bf16 matmuls with fp32 accumulation + int8 output quantization
-> rel err ~9e-3 vs fp32 ref (gate: 2e-2). All ScalarE activations stay
inside the single "exp_and_others" table set (exp, tanh, copy); sigmoid
is computed as tanh via sigma(x) = 0.5(1+tanh(x/2)) with the 0.5 factors
folded into the weights.
"""

import hashlib
from contextlib import ExitStack

import numpy as np
import ml_dtypes

import bass_rust
import concourse.bass as bass
import concourse.tile as tile
import concourse.mybir as mybir

BF16 = ml_dtypes.bfloat16
F32 = mybir.dt.float32
BF = mybir.dt.bfloat16
I8 = mybir.dt.int8

NCORES = 8
B = 32            # total batch
BC = 4            # batch per core
NREG = 196        # attention regions
NPAD = 256        # padded regions (2 chunks of 128 per batch element)
E = 512           # embed dim == hidden dim
G = 2048          # gate dim (4*H)
SEQ = 64
V = 32000
KCH = E // 128    # 4 k-chunks of the hidden dim
NVOC = (V + 511) // 512   # 63 vocab chunks of 512
VP = NVOC * 512           # 32256, int8 output padded to chunk multiple
MCH = (BC * SEQ + 127) // 128  # 2 row-chunks of the (t, b) dim

_ACT = mybir.ActivationFunctionType

# ---------------------------------------------------------------------------
# Workaround for a walrus codegen limit: an InstDrain may carry only one sync
# wait, but TileContext._drain_and_barrier attaches every outstanding proc's
# wait to one tail drain. Split the waits across a chain of drains.


def _split_drain_and_barrier(self, tick_clock, wait_clock):
    nc = self.nc
    drain_inst = nc.sync.drain()
    wait_clock.add_sem_waits(
        drain_inst.ins, bass_rust.ScopedClock({None: tick_clock.global_clock})
    )
    si = drain_inst.ins.sync_info
    if si is not None and si.on_wait is not None and len(si.on_wait) > 1:
        waits = list(si.on_wait)
        si.on_wait = waits[:1]
        for w in waits[1:]:
            d2 = nc.sync.drain()
            d2.ins.sync_info = bass_rust.SyncInfo(on_wait=[w], on_update=[])
    nc.all_engine_barrier()
    assert self.sems is not None
    popped = nc._tile_sem_poison_stack.pop()
    assert popped is self._sem_poison
    nc.clear_and_free_semaphores(list(self.sems.allocated().values()))
    nc.all_engine_barrier()


tile.TileContext._drain_and_barrier = _split_drain_and_barrier


# This walrus build rejects ANY instruction carrying more than one sync wait
# ("Too many sync wait commands"), while Tile freely attaches one wait per
# producer. General fix: post-process the BIR JSON, hoisting excess waits
# onto single-wait Drain instructions inserted just before the offender on
# the same engine.
def _split_multiwait_bir(bir_bytes):
    import orjson
    d = orjson.loads(bir_bytes)
    ctr = 0
    for f in d["functions"]:
        for bb in f["blocks"]:
            insts = bb.get("instructions")
            if not insts:
                continue
            out = []
            changed = False
            for inst in insts:
                si = inst.get("sync_info")
                waits = (si or {}).get("on_wait") or []
                cap = 2 if inst.get("opcode") == "EventSemaphore" else 1
                if len(waits) > cap:
                    changed = True
                    for w in waits[:-cap]:
                        ctr += 1
                        out.append({
                            "engine": inst["engine"],
                            "ins": [],
                            "name": f"I-mwsplit-{ctr}",
                            "opcode": "Drain",
                            "outs": [],
                            "sync_info": {"on_update": [], "on_wait": [w]},
                        })
                    si["on_wait"] = waits[-cap:]
                out.append(inst)
            if changed:
                bb["instructions"] = out
    return orjson.dumps(d)


from concourse import bass_utils  # noqa: E402
from concourse import bass2jax as _bass2jax  # noqa: E402

_orig_compile_bir_kernel = bass_utils.compile_bir_kernel


def _patched_compile_bir_kernel(bir_json, tmpdir, neff_name="file.neff"):
    return _orig_compile_bir_kernel(_split_multiwait_bir(bir_json), tmpdir,
                                    neff_name)


bass_utils.compile_bir_kernel = _patched_compile_bir_kernel
_bass2jax.compile_bir_kernel = _patched_compile_bir_kernel
# ---------------------------------------------------------------------------


def build_program(consts, seq=SEQ):
    """Trace the per-core Tile program. Weight arrays in `consts` are baked
    into the NEFF as Const tensors. Returns the Bass module."""
    nc = bass.Bass("TRN2", target_bir_lowering=False, debug=False,
                   num_devices=NCORES)

    dt = nc.dram_tensor
    fP0_d = dt("fP0", [128, BC * E], BF, kind="ExternalInput")
    fP1_d = dt("fP1", [NREG - 128, BC * E], BF, kind="ExternalInput")
    h0T_d = dt("h0T", [128, 4 * KCH], BF, kind="ExternalInput")
    c0_d = dt("c0", [BC, E], F32, kind="ExternalInput")
    embT_d = dt("embT", [128, KCH * 2 * 128], BF, kind="ExternalInput")
    wcgT_d = nc.inline_tensor(consts["wcgT"], "wcgT")
    whhT_d = nc.inline_tensor(consts["whhT"], "whhT")
    outWT_d = nc.inline_tensor(consts["outWT"], "outWT")
    outb_d = nc.inline_tensor(consts["outb"], "outb")
    eye4_d = nc.inline_tensor(consts["eye4"], "eye4")
    eye128_d = nc.inline_tensor(consts["eye128"], "eye128")
    gwT_d = nc.inline_tensor(consts["gwT"], "gwT")
    gbias_d = nc.inline_tensor(consts["gbias"], "gbias")
    out_d = dt("out", [BC, seq, V], I8, kind="ExternalOutput")
    scal_d = dt("scales", [128, MCH * NVOC], F32, kind="ExternalOutput")

    with tile.TileContext(nc) as tc:
        _trace(nc, tc, seq,
               fP0_d.ap(), fP1_d.ap(), h0T_d.ap(), c0_d.ap(), embT_d.ap(),
               wcgT_d.ap(), whhT_d.ap(), outWT_d.ap(), outb_d.ap(),
               eye4_d.ap(), eye128_d.ap(), gwT_d.ap(), gbias_d.ap(),
               out_d.ap(), scal_d.ap())
    return nc


def _trace(nc, tc, seq, fP0_d, fP1_d, h0T_d, c0_d, embT_d, wcgT_d, whhT_d,
           outWT_d, outb_d, eye4_d, eye128_d, gwT_d, gbias_d, out_d, scal_d):
    ht_cols = 4 * (seq + 1)
    mm = nc.tensor.matmul

    with ExitStack() as ctx:
        # ---------------- persistent SBUF (spans both phases) --------------
        pers = ctx.enter_context(tc.tile_pool(name="pers", bufs=1))
        fT = pers.tile([128, KCH * BC * NREG], BF, tag="fT")
        fP = pers.tile([128, 2 * BC * E], BF, tag="fP")
        wcgT = pers.tile([128, KCH * G], BF, tag="wcgT")
        whhT = pers.tile([128, KCH * G], BF, tag="whhT")
        HT = pers.tile([128, KCH * ht_cols], BF, tag="HT")  # col=ht_cols*k+4t+b
        cst = pers.tile([BC, E], F32, tag="cst")
        eye4 = pers.tile([BC, BC], BF, tag="eye4")
        eye128 = pers.tile([128, 128], BF, tag="eye128")
        onescol = pers.tile([128, 1], BF, tag="onescol")
        # current h, transposed, with stride-2 columns (col = 8k + 2b) so each
        # single-column matmul lhsT is 4-byte aligned in bf16
        hT2 = pers.tile([128, 8 * KCH], BF, tag="hT2")
        attn_bf = pers.tile([128, NPAD], BF, tag="attn_bf")
        BD = pers.tile([128, 4 * 2 * BC], BF, tag="BD")
        ctxT = pers.tile([128, 4 * KCH], BF, tag="ctxT")
        scales = pers.tile([128, MCH * NVOC], F32, tag="scales")

        # fP is shipped without the region padding: rc=0 rows fully, rc=1
        # only the 68 valid rows. Pad rows are zeroed (attention weights for
        # pad regions are zero, but NaN garbage would poison 0*x in the PE).
        fP_4d = fP[:].rearrange("p (b rc e) -> p b rc e", b=BC, rc=2)
        nc.sync.dma_start(
            fP_4d[:, :, 0, :],
            fP0_d[:].rearrange("p (b e) -> p b e", b=BC))
        # zero the pad rows first (DVE partition base must be 32-aligned,
        # so clear [64:128] and let the fP1 DMA overwrite rows 64..67)
        for b in range(BC):
            nc.vector.memset(
                fP[64:128, 1024 * b + 512: 1024 * b + 1024], 0.0)
        nc.sync.dma_start(
            fP_4d[0:NREG - 128, :, 1, :],
            fP1_d[:].rearrange("p (b e) -> p b e", b=BC))
        nc.sync.dma_start(wcgT[:], wcgT_d[:])
        nc.sync.dma_start(whhT[:], whhT_d[:])
        nc.sync.dma_start(cst[:], c0_d[:])
        nc.sync.dma_start(eye4[:], eye4_d[:])
        nc.sync.dma_start(eye128[:], eye128_d[:])
        nc.sync.dma_start(
            HT[:].rearrange("p (k c) -> p k c", k=KCH)[:, :, 0:4],
            h0T_d[:].rearrange("p (k c) -> p k c", k=KCH))
        nc.sync.dma_start(
            hT2[:].rearrange("p (k b two) -> p k b two", k=KCH, two=2)
            [:, :, :, 0:1],
            h0T_d[:].rearrange("p (k b one) -> p k b one", k=KCH, one=1))
        nc.vector.memset(onescol[:], 1.0)
        nc.vector.memset(attn_bf[:, NREG:NPAD], 0.0)

        # fT derived on device from fP (saves shipping the second features
        # layout over the slow host link): fT[:, NREG*(BC*k+b)+128*rc ...] =
        # 0.5 * transpose(fP block for (b, rc, k)).  The 0.5 is the sigma
        # x2-h folding factor (see the LSTM cell comment below).
        with tc.tile_pool(name="ps_ft", bufs=2, space="PSUM") as ps_ft:
            for k in range(KCH):
                for b in range(BC):
                    for rc in range(2):
                        w = 128 if rc == 0 else NREG - 128
                        tp = ps_ft.tile([128, 128], BF, tag="ftp")
                        nc.tensor.transpose(
                            tp[:],
                            fP[:, 1024 * b + 512 * rc + 128 * k:
                               1024 * b + 512 * rc + 128 * k + 128],
                            eye128[:])
                        dst = fT[:, NREG * (BC * k + b) + 128 * rc:
                                 NREG * (BC * k + b) + 128 * rc + w]
                        if (b + rc) % 2 == 0:
                            nc.scalar.activation(dst, tp[:, 0:w], _ACT.Copy,
                                                 scale=0.5)
                        else:
                            nc.vector.tensor_scalar_mul(dst, tp[:, 0:w], 0.5)

        # phase-2 shared resources (vocab projection), usable both inside the
        # recurrence (idle-PE interleave) and in the tail loop
        ones1 = pers.tile([1, 128], BF, tag="ones1")
        nc.vector.memset(ones1[:], 1.0)
        outb_sb = pers.tile([1, V], BF, tag="outb_sb")
        nc.sync.dma_start(outb_sb[:], outb_d[:])

        # G_emb computed on device (ships emb.T [128, 1024] instead of the
        # 4x larger gate-space gemb): G_emb = emb @ G_W.T + G_bias, stored
        # bf16 as two row-halves gemb_sb[:, 2048h + g] with psum row
        # r = 4*t_rel + b, t = 32h + t_rel.
        embT = pers.tile([128, KCH * 2 * 128], BF, tag="embT")
        gwT = pers.tile([128, KCH * G], BF, tag="gwT")
        gemb_sb = pers.tile([128, 2 * G], BF, tag="gemb_sb")
        gbias_sb = pers.tile([1, G], BF, tag="gbias_sb")
        nc.sync.dma_start(embT[:], embT_d[:])
        nc.sync.dma_start(gwT[:], gwT_d[:])
        nc.sync.dma_start(gbias_sb[:], gbias_d[:])
        with tc.tile_pool(name="ps_ge", bufs=1, space="PSUM") as ps_ge:
            for h in range(2):
                gp = ps_ge.tile([128, G], F32, tag="gep")
                for n in range(4):
                    gsl = slice(512 * n, 512 * n + 512)
                    for k in range(KCH):
                        mm(gp[:, gsl],
                           embT[:, 256 * k + 128 * h: 256 * k + 128 * h + 128],
                           gwT[:, G * k + 512 * n: G * k + 512 * n + 512],
                           start=(k == 0), stop=False)
                    mm(gp[:, gsl], ones1[0:1, 0:128], gbias_sb[0:1, gsl],
                       start=False, stop=True)
                if h == 0:
                    nc.scalar.copy(gemb_sb[:, 0:G], gp[:])
                else:
                    nc.vector.tensor_copy(gemb_sb[:, G:2 * G], gp[:])
        wsb = ctx.enter_context(tc.tile_pool(name="wsb", bufs=12))
        osb = ctx.enter_context(tc.tile_pool(name="osb", bufs=4))
        qsb = ctx.enter_context(tc.tile_pool(name="qsb", bufs=4))
        nvoc = NVOC
        mch = MCH

        def emit_p2(m, n, ps_pool, eng_flip):
            # deprioritize against the recurrence chain for engine contention
            tc.cur_priority += 50000
            _emit_p2_body(m, n, ps_pool, eng_flip)
            tc.cur_priority -= 50000

        def _emit_p2_body(m, n, ps_pool, eng_flip):
            nw = min(512, V - 512 * n)
            mr = min(128, BC * seq - 128 * m)
            wts = []
            for k in range(KCH):
                wt = wsb.tile([128, 512], BF, tag="wt")
                nc.sync.dma_start(
                    wt[:, 0:nw],
                    outWT_d[128 * k: 128 * k + 128, 512 * n: 512 * n + nw])
                wts.append(wt)
            ps = ps_pool.tile([128, 512], F32, tag="po")
            for k in range(KCH):
                mm(ps[0:mr, 0:nw],
                   HT[:, ht_cols * k + 4 + 128 * m:
                      ht_cols * k + 4 + 128 * m + mr],
                   wts[k][:, 0:nw],
                   start=(k == 0), stop=False)
            mm(ps[0:mr, 0:nw], ones1[0:1, 0:mr],
               outb_sb[0:1, 512 * n: 512 * n + nw],
               start=False, stop=True)
            # int8-quantize per output row: q = round(x * 127/absmax(row)),
            # scale[row] = absmax/127 shipped alongside (HW convert is
            # round-to-nearest with saturation).
            mx = qsb.tile([128, 1], F32, tag="qmx")
            nc.vector.tensor_reduce(mx[0:mr], ps[0:mr, 0:nw],
                                    axis=mybir.AxisListType.X,
                                    op=mybir.AluOpType.max,
                                    apply_absolute_value=True)
            rq = qsb.tile([128, 1], F32, tag="qrq")
            nc.vector.reciprocal(rq[0:mr], mx[0:mr])
            nc.vector.tensor_scalar_mul(rq[0:mr], rq[0:mr], 127.0)
            nc.vector.tensor_scalar_mul(
                scales[0:mr, nvoc * m + n: nvoc * m + n + 1],
                mx[0:mr], 1.0 / 127.0)
            ob = osb.tile([128, 512], I8, tag="ob")
            if eng_flip:
                nc.scalar.activation(ob[0:mr, 0:nw], ps[0:mr, 0:nw],
                                     _ACT.Copy, scale=rq[0:mr])
            else:
                nc.vector.tensor_scalar_mul(ob[0:mr, 0:nw], ps[0:mr, 0:nw],
                                            rq[0:mr])
            dst = out_d[:, 32 * m: 32 * m + mr // 4, 512 * n: 512 * n + nw]
            nc.sync.dma_start(dst.rearrange("b t v -> t b v"), ob[0:mr, 0:nw])

        # ---------------- recurrence ----------------
        with ExitStack() as rctx:
            sb = rctx.enter_context(tc.tile_pool(name="sb", bufs=2))
            ps_sc = rctx.enter_context(
                tc.tile_pool(name="ps_sc", bufs=1, space="PSUM"))
            ps_tp = rctx.enter_context(
                tc.tile_pool(name="ps_tp", bufs=1, space="PSUM"))
            ps_g = rctx.enter_context(
                tc.tile_pool(name="ps_g", bufs=1, space="PSUM"))
            ps_oi = rctx.enter_context(
                tc.tile_pool(name="ps_oi", bufs=1, space="PSUM"))
            p2_done = 0  # m=0 vocab chunks emitted inside the recurrence

            # scores psum: batch b's scores live in row 32*b (col-group
            # tile_position); untouched rows stay 0 from this one memset.
            psum_s = ps_sc.tile([128, 512], F32, tag="ps_s")
            nc.vector.memset(psum_s[:], 0.0)

            for t in range(seq):
                hc = 4 * t

                # scores row for batch b at partition 32b:
                # psum_s[32b, n] = <h_b, F[b,n,:]>
                for b in range(BC):
                    for k in range(KCH):
                        mm(psum_s[32 * b: 32 * b + 1, 0:NREG],
                           hT2[:, 8 * k + 2 * b: 8 * k + 2 * b + 1],
                           fT[:, BC * NREG * k + NREG * b:
                              BC * NREG * k + NREG * (b + 1)],
                           start=(k == 0), stop=(k == KCH - 1),
                           tile_position=(0, 32 * b))

                # gates part 1: h @ Whh.T + G_emb  (PE work hiding softmax).
                # G_emb rows for step t come from gemb_sb via an eye128
                # column-select (rows 4*(t%32)..+4 of half t//32).
                gps = ps_g.tile([BC, G], F32, tag="gps")
                tr4 = 4 * (t % 32)
                gh = G * (t // 32)
                for n in range(4):
                    gsl = slice(512 * n, 512 * n + 512)
                    for k in range(KCH):
                        mm(gps[:, gsl],
                           HT[:, ht_cols * k + hc: ht_cols * k + hc + 4],
                           whhT[:, G * k + 512 * n: G * k + 512 * n + 512],
                           start=(k == 0), stop=False)
                    mm(gps[:, gsl], eye128[:, tr4: tr4 + 4],
                       gemb_sb[:, gh + 512 * n: gh + 512 * n + 512],
                       start=False, stop=False)

                # softmax along the free dim, rows {0,32,64,96} meaningful
                mx = sb.tile([128, 1], F32, tag="mx")
                nc.vector.reduce_max(mx[:], psum_s[:, 0:NREG],
                                     axis=mybir.AxisListType.X)
                nmx = sb.tile([128, 1], F32, tag="nmx")
                nc.vector.tensor_scalar_mul(nmx[:], mx[:], -1.0)
                ssum = sb.tile([128, 1], F32, tag="ssum")
                nc.scalar.activation(attn_bf[:, 0:NREG], psum_s[:, 0:NREG], _ACT.Exp,
                                     bias=nmx[:], scale=1.0, accum_out=ssum[:])
                rinv = sb.tile([128, 1], F32, tag="rinv")
                nc.vector.reciprocal(rinv[:], ssum[:])
                nc.vector.tensor_scalar_mul(attn_bf[:, 0:NREG],
                                            attn_bf[:, 0:NREG], rinv[:])

                # attn.T via row-wise PE transposes -> block-diag scatter
                atp = ps_tp.tile([128, 4 * BC], BF, tag="tpb")
                for b in range(BC):
                    for k2 in range(2):
                        c2 = 2 * b + k2
                        mm(atp[:, 2 * c2: 2 * c2 + 1],
                           attn_bf[32 * b: 32 * b + 1,
                                   128 * k2: 128 * (k2 + 1)],
                           onescol[32 * b: 32 * b + 1, 0:1],
                           is_transpose=True, tile_position=(32 * b, 0))
                nc.vector.memset(BD[:], 0.0)
                # dst col 4*(2b+k2)+b = 9b+4k2, src col 2*(2b+k2) = 4b+2k2:
                # both affine in (b, k2) -> a single strided-AP copy
                bd_dst = bass.AP(BD.tensor, BD.offset,
                                 [BD.ap[0], [9, BC], [4, 2]])
                bd_src = bass.AP(atp.tensor, atp.offset,
                                 [atp.ap[0], [4, BC], [2, 2]])
                nc.scalar.copy(bd_dst, bd_src)

                # context transposed: ctxT[e, b]
                cps = ps_tp.tile([128, 4 * KCH], F32, tag="cps")
                for m in range(KCH):
                    for c2 in range(2 * BC):
                        mm(cps[:, 4 * m: 4 * m + 4],
                           fP[:, 512 * c2 + 128 * m: 512 * c2 + 128 * m + 128],
                           BD[:, 4 * c2: 4 * c2 + 4],
                           start=(c2 == 0), stop=(c2 == 2 * BC - 1))
                nc.scalar.copy(ctxT[:], cps[:])

                # gates part 2: ctx @ W_cg.T
                for n in range(4):
                    gsl = slice(512 * n, 512 * n + 512)
                    for k in range(KCH):
                        mm(gps[:, gsl],
                           ctxT[:, 4 * k: 4 * k + 4],
                           wcgT[:, G * k + 512 * n: G * k + 512 * n + 512],
                           start=False, stop=(k == KCH - 1))

                # vocab-projection chunks for rows t<32 interleave into the
                # idle PE window left by the elementwise chain (also keeps
                # the PE p-state warm)
                if seq == SEQ and t >= 33:
                    quota = min(nvoc, 2 * (t - 32))
                    while p2_done < quota:
                        emit_p2(0, p2_done, ps_oi, p2_done % 2 == 0)
                        p2_done += 1

                # LSTM cell via tanh-only activations (one ACT table set).
                # sigma(x) = 0.5(1+tanh(x/2)); h is stored as 2h with the
                # 0.5 factors folded into fT/whhT/outWT/h0T on the host, so
                # each sigma-multiply fuses into one scalar_tensor_tensor:
                #   u0 = (1+th_f)*c = 2*sig(f)*c
                #   u1 = (1+th_i)*tg = 2*sig(i)*tanh(g)
                #   v = u0+u1 = 2*c2;  c <- 0.5v;  tanh(c2) = Tanh(0.5*v)
                #   h2x2 = (1+th_o)*tanh(c2) = 2*h2
                mlop = mybir.AluOpType.mult
                adop = mybir.AluOpType.add
                thif = sb.tile([BC, 1024], F32, tag="thif")
                nc.scalar.activation(thif[:], gps[:, 0:1024], _ACT.Tanh,
                                     scale=0.5)
                tg = sb.tile([BC, 512], F32, tag="tg")
                nc.scalar.activation(tg[:], gps[:, 1024:1536], _ACT.Tanh)
                tho = sb.tile([BC, 512], F32, tag="tho")
                nc.scalar.activation(tho[:], gps[:, 1536:2048], _ACT.Tanh,
                                     scale=0.5)
                u0 = sb.tile([BC, 512], F32, tag="u0")
                nc.vector.scalar_tensor_tensor(u0[:], thif[:, 512:1024], 1.0,
                                               cst[:], adop, mlop)
                u1 = sb.tile([BC, 512], F32, tag="u1")
                nc.vector.scalar_tensor_tensor(u1[:], thif[:, 0:512], 1.0,
                                               tg[:], adop, mlop)
                v2c = sb.tile([BC, 512], F32, tag="v2c")
                nc.vector.tensor_add(v2c[:], u0[:], u1[:])
                tc2 = sb.tile([BC, 512], F32, tag="tc2")
                nc.scalar.activation(tc2[:], v2c[:], _ACT.Tanh, scale=0.5)
                nc.vector.tensor_scalar_mul(cst[:], v2c[:], 0.5)
                h2 = sb.tile([BC, 512], BF, tag="h2")
                nc.vector.scalar_tensor_tensor(h2[:], tho[:], 1.0, tc2[:],
                                               adop, mlop)

                # h2.T -> HT col group t+1
                hps = ps_tp.tile([128, 4 * KCH], BF, tag="tpb")
                for m in range(KCH):
                    nc.tensor.transpose(hps[:, 4 * m: 4 * m + 4],
                                        h2[0:4, 128 * m: 128 * m + 128],
                                        eye4[:])
                ht_dst = bass.AP(HT.tensor, HT.offset + 4 * (t + 1),
                                 [HT.ap[0], [ht_cols, KCH], [1, 4]])
                nc.scalar.copy(ht_dst, hps[:].rearrange(
                    "p (m c) -> p m c", m=KCH))
                h2_dst = bass.AP(hT2.tensor, hT2.offset,
                                 [hT2.ap[0], [8, KCH], [2, 4]])
                nc.vector.tensor_copy(h2_dst, hps[:].rearrange(
                    "p (m c) -> p m c", m=KCH))

        # ------- phase 2 tail: remaining vocab-projection chunks -----------
        import os
        if os.environ.get("K_SKIP_P2"):
            return
        with ExitStack() as ctx2:
            ps_o2 = ctx2.enter_context(
                tc.tile_pool(name="ps_o2", bufs=4, space="PSUM"))
            rest = []
            if seq == SEQ:
                try:
                    rest += [(0, n) for n in range(p2_done, nvoc)]
                except NameError:
                    rest += [(0, n) for n in range(nvoc)]
                rest += [(m, n) for m in range(1, mch) for n in range(nvoc)]
            else:
                rest += [(m, n) for m in range(mch) for n in range(nvoc)]
            for i, (m, n) in enumerate(rest):
                emit_p2(m, n, ps_o2, i % 2 == 0)
        nc.sync.dma_start(scal_d[:], scales[:])


def host_fold_weights(inputs):
    """Fold the weight tensors into the const arrays baked into the NEFF."""
    f32 = np.float32
    reshape_W = np.asarray(inputs["reshape_W"], f32)
    reshape_b = np.asarray(inputs["reshape_b"], f32)
    Wih = np.asarray(inputs["lstm_Wih"], f32)
    Whh = np.asarray(inputs["lstm_Whh"], f32)
    bih = np.asarray(inputs["lstm_bih"], f32)
    bhh = np.asarray(inputs["lstm_bhh"], f32)
    out_W = np.asarray(inputs["out_W"], f32)
    out_b = np.asarray(inputs["out_b"], f32)

    R1, R2 = reshape_W[:, :E], reshape_W[:, E:]
    W_cg = Wih @ R1
    G_W = Wih @ R2
    G_bias = reshape_b @ Wih.T + bih + bhh

    def kmajor(x):   # [512, cols] -> [128, 4*cols], col = cols*k + c
        c = x.shape[1]
        return np.ascontiguousarray(
            x.reshape(KCH, 128, c).transpose(1, 0, 2).reshape(128, KCH * c))

    return {
        "wcgT": kmajor(W_cg.T).astype(BF16),
        "whhT": kmajor(0.5 * Whh.T).astype(BF16),
        "outWT": np.ascontiguousarray(0.5 * out_W.T).astype(BF16),
        "outb": out_b.reshape(1, V).astype(BF16),
        "eye4": np.eye(BC, dtype=BF16),
        "eye128": np.eye(128, dtype=BF16),
        "gwT": kmajor(G_W.T).astype(BF16),
        "gbias": G_bias.reshape(1, G).astype(BF16),
    }


def host_prep_data(inputs, seq=SEQ):
    """Per-call data inputs -> the 8 per-core in_maps (weights excluded)."""
    f32 = np.float32
    features = np.asarray(inputs["features"], f32)
    captions = np.asarray(inputs["captions"])
    embed_W = np.asarray(inputs["embed_W"], f32)
    init_h_W = np.asarray(inputs["init_h_W"], f32)
    init_h_b = np.asarray(inputs["init_h_b"], f32)
    init_c_W = np.asarray(inputs["init_c_W"], f32)
    init_c_b = np.asarray(inputs["init_c_b"], f32)

    emb = embed_W[captions] * np.sqrt(f32(E))           # [B, S, E]
    fmean = features.mean(axis=1)
    h0 = fmean @ init_h_W.T + init_h_b
    c0 = fmean @ init_c_W.T + init_c_b

    in_maps = []
    for c in range(NCORES):
        bs = slice(BC * c, BC * (c + 1))
        Fc = features[bs]
        fP0 = Fc[:, :128].transpose(1, 0, 2).reshape(128, BC * E)
        fP1 = Fc[:, 128:NREG].transpose(1, 0, 2).reshape(NREG - 128, BC * E)
        h0T = (2.0 * h0[bs].T.reshape(KCH, 128, BC)
               .transpose(1, 0, 2).reshape(128, KCH * BC))
        # embT col = 256k + 128h + 4*t_rel + b  (t = 32h + t_rel)
        embT = (emb[bs, :seq].transpose(2, 1, 0)        # [e, t, b]
                .reshape(KCH, 128, 2, 32, BC)
                .transpose(1, 0, 2, 3, 4).reshape(128, KCH * 2 * 128))
        in_maps.append({
            "fP0": np.ascontiguousarray(fP0).astype(BF16),
            "fP1": np.ascontiguousarray(fP1).astype(BF16),
            "h0T": np.ascontiguousarray(h0T).astype(BF16),
            "c0": np.ascontiguousarray(c0[bs]),
            "embT": np.ascontiguousarray(embT).astype(BF16),
        })
    return in_maps


def assemble_output(outs):
    """Dequantize int8+scales global outputs -> [B, SEQ, V] float32.

    Output row r of row-chunk m maps to (t = 32m + r//4, b_local = r%4);
    scales column index is m*NVOC + n for vocab chunk n.
    """
    O = outs["out"]          # [B, SEQ, V] int8   (B = NCORES*BC)
    S = outs["scales"]       # [NCORES*128, MCH*NVOC] f32
    Sb = (S.reshape(NCORES, 128, MCH, NVOC)
          .reshape(NCORES, 32, BC, MCH, NVOC)
          .transpose(0, 2, 3, 1, 4)        # [core, b, m, t_rel, n]
          .reshape(B, SEQ, NVOC))
    out = np.empty((B, SEQ, V), np.float32)
    nfull = V // 512                       # full 512-wide chunks
    vf = nfull * 512
    out[:, :, :vf] = (O[:, :, :vf].reshape(B, SEQ, nfull, 512)
                      * Sb[:, :, :nfull, None]).reshape(B, SEQ, vf)
    out[:, :, vf:] = O[:, :, vf:] * Sb[:, :, nfull, None]
    return out


# ---------------------------------------------------------------------------
# Cached PJRT runner: jit the bass program ONCE per weight-set; later calls
# only pay input transfer + execute + output fetch.

class _Bundle:
    def __init__(self, nc):
        import jax
        from jax.experimental.shard_map import shard_map
        from jax.sharding import Mesh, PartitionSpec

        _bass2jax.install_neuronx_cc_hook()
        self.nc = nc
        partition_name = (nc.partition_id_tensor.name
                          if nc.partition_id_tensor else None)
        in_names, out_names, out_avals = [], [], []
        for alloc in nc.m.functions[0].allocations:
            if not isinstance(alloc, mybir.MemoryLocationSet):
                continue
            name = alloc.memorylocations[0].name
            if alloc.kind == "ExternalInput":
                if name != partition_name:
                    in_names.append(name)
            elif alloc.kind == "ExternalOutput":
                out_names.append(name)
                out_avals.append(jax.core.ShapedArray(
                    tuple(alloc.tensor_shape), mybir.dt.np(alloc.dtype)))
        self.in_names, self.out_names = in_names, out_names
        self.out_avals = out_avals
        bind_in_names = list(in_names)
        if partition_name is not None:
            bind_in_names.append(partition_name)

        def _body(*args):
            operands = list(args)
            if partition_name is not None:
                operands.append(_bass2jax.partition_id_tensor())
            outs = _bass2jax._bass_exec_p.bind(
                *operands,
                out_avals=tuple(out_avals),
                in_names=tuple(bind_in_names),
                out_names=tuple(out_names),
                lowering_input_output_aliases=(),
                sim_require_finite=True,
                sim_require_nnan=True,
                nc=nc,
            )
            return tuple(outs)

        devices = jax.devices()[:NCORES]
        assert len(devices) == NCORES
        mesh = Mesh(np.asarray(devices), ("core",))
        P = PartitionSpec
        self.jitfn = jax.jit(shard_map(
            _body, mesh=mesh,
            in_specs=(P("core"),) * len(in_names),
            out_specs=(P("core"),) * len(out_names),
            check_rep=False))

    def run(self, in_maps):
        """in_maps (host numpy, one dict per core) -> global outputs on host
        (dict name -> [NCORES*dim0, ...])."""
        import jax
        concat_in = [
            np.concatenate([m[name] for m in in_maps], axis=0)
            for name in self.in_names
        ]
        out_arrs = self.jitfn(*concat_in)
        # fetch all output shards concurrently (the axon link serializes a
        # single np.asarray shard-by-shard at ~2/3 of its aggregate rate)
        shard_data, meta = [], []
        for name, arr in zip(self.out_names, out_arrs):
            for s in arr.addressable_shards:
                shard_data.append(s.data)
                meta.append((name, s.index))
        vals = jax.device_get(shard_data)
        result = {}
        for i, (name, arr) in enumerate(zip(self.out_names, out_arrs)):
            g = np.empty(
                (NCORES * self.out_avals[i].shape[0],
                 *self.out_avals[i].shape[1:]),
                self.out_avals[i].dtype)
            result[name] = g
        for (name, idx), v in zip(meta, vals):
            result[name][idx] = v
        return result


_bundle_cache = {}


def get_bundle(inputs):
    key = hashlib.blake2b(
        b"".join(np.ascontiguousarray(np.asarray(inputs[k], np.float32))
                 .tobytes()
                 for k in ("reshape_W", "reshape_b", "lstm_Wih", "lstm_Whh",
                           "lstm_bih", "lstm_bhh", "out_W", "out_b")),
        digest_size=16).hexdigest()
    if key not in _bundle_cache:
        consts = host_fold_weights(inputs)
        nc = build_program(consts, SEQ)
        _bundle_cache[key] = _Bundle(nc)
    return _bundle_cache[key]


def kernel(**inputs) -> np.ndarray:
    bundle = get_bundle(inputs)
    in_maps = host_prep_data(inputs, SEQ)
    outs = bundle.run(in_maps)
    return assemble_output(outs)                 # [32, SEQ, V] f32


if __name__ == "__main__":
    import reference as refmod
    inputs = {k: np.asarray(v) for k, v in refmod.setup_inputs().items()}
    expected = np.asarray(refmod.reference(**inputs))
    got = kernel(**inputs)
    err = np.abs(got - expected).max() / np.abs(expected).max()
    l2 = np.linalg.norm((got - expected).ravel()) / np.linalg.norm(expected.ravel())
    print(f"Relative error: {err:.3e} (l2 {l2:.3e})")


# revision 39
# speedup vs baseline: 1.0575x; 1.0575x over previous
"""DecoderLSTM (attention + LSTM + vocab projection) on 8 Trainium2 NeuronCores.

Strategy (data-parallel over batch, no collectives):
  - Each of the 8 cores owns 4 of the 32 batch elements and runs the full
    64-step attention-LSTM recurrence for them in bf16 (fp32 cell state),
    storing h_t transposed in SBUF.
  - The vocab projection (90% of FLOPs) is hoisted out of the recurrence:
    one dense [256,512]@[512,32000] matmul per core, streaming out_W.T
    from HBM, partially interleaved into the recurrence's idle PE windows.
  - Algebraic folds done on host (numpy): the embedding gather, h0/c0 init,
    reshape_W folded into the LSTM input weights (W_cg = Wih @ R1), and the
    per-step embedding contribution G_emb[t] = emb_t @ (Wih R2).T + biases.

Wall-clock engineering (the axon tunnel runs at ~40 MB/s, so bytes moved
per call dominate end-to-end latency):
  - All weight tensors (wcgT/whhT/outWT/outb/gwT/gbias + identities) are
    baked into the NEFF as Const tensors (inline_tensor): they ride in the
    executable at load time and are NOT re-transferred per call. kernel()
    re-specializes (retrace + recompile) if the weight values change.
  - The PJRT executable is jitted ONCE per weight-set and cached; calls
    after the first skip trace/lower/compile entirely.
  - No zero output buffers are shipped (the kernel writes every output
    element, so uninitialized XLA output buffers are fine).
  - Per-call H2D is only the data-dependent inputs (~8.7 MB total):
    unpadded features (fP0/fP1), transposed token embeddings (embT), and
    h0/c0. fT (the transposed features layout) and G_emb (the gate-space
    embedding contribution) are derived on device.
  - The logits cross the tunnel int8-quantized with per-row scales
    (absmax/127, hardware round-to-nearest), ~66 MB instead of 262 MB
    f32; the host dequantizes. Output shards are fetched concurrently
    (jax.device_get on the shard list) — a lone np.asarray walks shards
    serially at ~2/3 the rate.

Numerics: bf16 matmuls with fp32 accumulation + int8 output quantization
-> rel err ~9e-3 vs fp32 ref (gate: 2e-2). All ScalarE activations stay
inside the single "exp_and_others" table set (exp, tanh, copy); sigmoid
is computed as tanh via sigma(x) = 0.5(1+tanh(x/2)) with the 0.5 factors
folded into the weights.
"""

import hashlib
from contextlib import ExitStack

import numpy as np
import ml_dtypes

import bass_rust
import concourse.bass as bass
import concourse.tile as tile
import concourse.mybir as mybir

BF16 = ml_dtypes.bfloat16
F32 = mybir.dt.float32
BF = mybir.dt.bfloat16
I8 = mybir.dt.int8

NCORES = 8
B = 32            # total batch
BC = 4            # batch per core
NREG = 196        # attention regions
NPAD = 256        # padded regions (2 chunks of 128 per batch element)
E = 512           # embed dim == hidden dim
G = 2048          # gate dim (4*H)
SEQ = 64
V = 32000
KCH = E // 128    # 4 k-chunks of the hidden dim
NVOC = (V + 511) // 512   # 63 vocab chunks of 512
VP = NVOC * 512           # 32256, int8 output padded to chunk multiple
MCH = (BC * SEQ + 127) // 128  # 2 row-chunks of the (t, b) dim

_ACT = mybir.ActivationFunctionType

# ---------------------------------------------------------------------------
# Workaround for a walrus codegen limit: an InstDrain may carry only one sync
# wait, but TileContext._drain_and_barrier attaches every outstanding proc's
# wait to one tail drain. Split the waits across a chain of drains.


def _split_drain_and_barrier(self, tick_clock, wait_clock):
    nc = self.nc
    drain_inst = nc.sync.drain()
    wait_clock.add_sem_waits(
        drain_inst.ins, bass_rust.ScopedClock({None: tick_clock.global_clock})
    )
    si = drain_inst.ins.sync_info
    if si is not None and si.on_wait is not None and len(si.on_wait) > 1:
        waits = list(si.on_wait)
        si.on_wait = waits[:1]
        for w in waits[1:]:
            d2 = nc.sync.drain()
            d2.ins.sync_info = bass_rust.SyncInfo(on_wait=[w], on_update=[])
    nc.all_engine_barrier()
    assert self.sems is not None
    popped = nc._tile_sem_poison_stack.pop()
    assert popped is self._sem_poison
    nc.clear_and_free_semaphores(list(self.sems.allocated().values()))
    nc.all_engine_barrier()


tile.TileContext._drain_and_barrier = _split_drain_and_barrier


# This walrus build rejects ANY instruction carrying more than one sync wait
# ("Too many sync wait commands"), while Tile freely attaches one wait per
# producer. General fix: post-process the BIR JSON, hoisting excess waits
# onto single-wait Drain instructions inserted just before the offender on
# the same engine.
def _split_multiwait_bir(bir_bytes):
    import orjson
    d = orjson.loads(bir_bytes)
    ctr = 0
    for f in d["functions"]:
        for bb in f["blocks"]:
            insts = bb.get("instructions")
            if not insts:
                continue
            out = []
            changed = False
            for inst in insts:
                si = inst.get("sync_info")
                waits = (si or {}).get("on_wait") or []
                cap = 2 if inst.get("opcode") == "EventSemaphore" else 1
                if len(waits) > cap:
                    changed = True
                    for w in waits[:-cap]:
                        ctr += 1
                        out.append({
                            "engine": inst["engine"],
                            "ins": [],
                            "name": f"I-mwsplit-{ctr}",
                            "opcode": "Drain",
                            "outs": [],
                            "sync_info": {"on_update": [], "on_wait": [w]},
                        })
                    si["on_wait"] = waits[-cap:]
                out.append(inst)
            if changed:
                bb["instructions"] = out
    return orjson.dumps(d)


from concourse import bass_utils  # noqa: E402
from concourse import bass2jax as _bass2jax  # noqa: E402

_orig_compile_bir_kernel = bass_utils.compile_bir_kernel


def _patched_compile_bir_kernel(bir_json, tmpdir, neff_name="file.neff"):
    return _orig_compile_bir_kernel(_split_multiwait_bir(bir_json), tmpdir,
                                    neff_name)


bass_utils.compile_bir_kernel = _patched_compile_bir_kernel
_bass2jax.compile_bir_kernel = _patched_compile_bir_kernel
# ---------------------------------------------------------------------------


def build_program(consts, seq=SEQ):
    """Trace the per-core Tile program. Weight arrays in `consts` are baked
    into the NEFF as Const tensors. Returns the Bass module."""
    nc = bass.Bass("TRN2", target_bir_lowering=False, debug=False,
                   num_devices=NCORES)

    dt = nc.dram_tensor
    fP0_d = dt("fP0", [128, BC * E], BF, kind="ExternalInput")
    fP1_d = dt("fP1", [NREG - 128, BC * E], BF, kind="ExternalInput")
    h0T_d = dt("h0T", [128, 4 * KCH], BF, kind="ExternalInput")
    c0_d = dt("c0", [BC, E], F32, kind="ExternalInput")
    embT_d = dt("embT", [128, KCH * 2 * 128], BF, kind="ExternalInput")
    wcgT_d = nc.inline_tensor(consts["wcgT"], "wcgT")
    whhT_d = nc.inline_tensor(consts["whhT"], "whhT")
    outWT_d = nc.inline_tensor(consts["outWT"], "outWT")
    outb_d = nc.inline_tensor(consts["outb"], "outb")
    eye4_d = nc.inline_tensor(consts["eye4"], "eye4")
    eye128_d = nc.inline_tensor(consts["eye128"], "eye128")
    gwT_d = nc.inline_tensor(consts["gwT"], "gwT")
    gbias_d = nc.inline_tensor(consts["gbias"], "gbias")
    out_d = dt("out", [BC, seq, V], I8, kind="ExternalOutput")
    scal_d = dt("scales", [128, MCH * NVOC], F32, kind="ExternalOutput")

    with tile.TileContext(nc) as tc:
        _trace(nc, tc, seq,
               fP0_d.ap(), fP1_d.ap(), h0T_d.ap(), c0_d.ap(), embT_d.ap(),
               wcgT_d.ap(), whhT_d.ap(), outWT_d.ap(), outb_d.ap(),
               eye4_d.ap(), eye128_d.ap(), gwT_d.ap(), gbias_d.ap(),
               out_d.ap(), scal_d.ap())
    return nc


def _trace(nc, tc, seq, fP0_d, fP1_d, h0T_d, c0_d, embT_d, wcgT_d, whhT_d,
           outWT_d, outb_d, eye4_d, eye128_d, gwT_d, gbias_d, out_d, scal_d):
    ht_cols = 4 * (seq + 1)
    mm = nc.tensor.matmul

    with ExitStack() as ctx:
        # ---------------- persistent SBUF (spans both phases) --------------
        pers = ctx.enter_context(tc.tile_pool(name="pers", bufs=1))
        fT = pers.tile([128, KCH * BC * NREG], BF, tag="fT")
        fP = pers.tile([128, 2 * BC * E], BF, tag="fP")
        wcgT = pers.tile([128, KCH * G], BF, tag="wcgT")
        whhT = pers.tile([128, KCH * G], BF, tag="whhT")
        HT = pers.tile([128, KCH * ht_cols], BF, tag="HT")  # col=ht_cols*k+4t+b
        cst = pers.tile([BC, E], F32, tag="cst")
        eye4 = pers.tile([BC, BC], BF, tag="eye4")
        eye128 = pers.tile([128, 128], BF, tag="eye128")
        onescol = pers.tile([128, 1], BF, tag="onescol")
        # current h, transposed, with stride-2 columns (col = 8k + 2b) so each
        # single-column matmul lhsT is 4-byte aligned in bf16
        hT2 = pers.tile([128, 8 * KCH], BF, tag="hT2")
        attn_bf = pers.tile([128, NPAD], BF, tag="attn_bf")
        BD = pers.tile([128, 4 * 2 * BC], BF, tag="BD")
        ctxT = pers.tile([128, 4 * KCH], BF, tag="ctxT")
        scales = pers.tile([128, MCH * NVOC], F32, tag="scales")

        # fP is shipped without the region padding: rc=0 rows fully, rc=1
        # only the 68 valid rows. Pad rows are zeroed (attention weights for
        # pad regions are zero, but NaN garbage would poison 0*x in the PE).
        fP_4d = fP[:].rearrange("p (b rc e) -> p b rc e", b=BC, rc=2)
        nc.sync.dma_start(
            fP_4d[:, :, 0, :],
            fP0_d[:].rearrange("p (b e) -> p b e", b=BC))
        # zero the pad rows first (DVE partition base must be 32-aligned,
        # so clear [64:128] and let the fP1 DMA overwrite rows 64..67)
        for b in range(BC):
            nc.vector.memset(
                fP[64:128, 1024 * b + 512: 1024 * b + 1024], 0.0)
        nc.sync.dma_start(
            fP_4d[0:NREG - 128, :, 1, :],
            fP1_d[:].rearrange("p (b e) -> p b e", b=BC))
        nc.sync.dma_start(wcgT[:], wcgT_d[:])
        nc.sync.dma_start(whhT[:], whhT_d[:])
        nc.sync.dma_start(cst[:], c0_d[:])
        nc.sync.dma_start(eye4[:], eye4_d[:])
        nc.sync.dma_start(eye128[:], eye128_d[:])
        nc.sync.dma_start(
            HT[:].rearrange("p (k c) -> p k c", k=KCH)[:, :, 0:4],
            h0T_d[:].rearrange("p (k c) -> p k c", k=KCH))
        nc.sync.dma_start(
            hT2[:].rearrange("p (k b two) -> p k b two", k=KCH, two=2)
            [:, :, :, 0:1],
            h0T_d[:].rearrange("p (k b one) -> p k b one", k=KCH, one=1))
        nc.vector.memset(onescol[:], 1.0)
        nc.vector.memset(attn_bf[:, NREG:NPAD], 0.0)

        # fT derived on device from fP (saves shipping the second features
        # layout over the slow host link): fT[:, NREG*(BC*k+b)+128*rc ...] =
        # 0.5 * transpose(fP block for (b, rc, k)).  The 0.5 is the sigma
        # x2-h folding factor (see the LSTM cell comment below).
        with tc.tile_pool(name="ps_ft", bufs=2, space="PSUM") as ps_ft:
            for k in range(KCH):
                for b in range(BC):
                    for rc in range(2):
                        w = 128 if rc == 0 else NREG - 128
                        tp = ps_ft.tile([128, 128], BF, tag="ftp")
                        nc.tensor.transpose(
                            tp[:],
                            fP[:, 1024 * b + 512 * rc + 128 * k:
                               1024 * b + 512 * rc + 128 * k + 128],
                            eye128[:])
                        dst = fT[:, NREG * (BC * k + b) + 128 * rc:
                                 NREG * (BC * k + b) + 128 * rc + w]
                        if (b + rc) % 2 == 0:
                            nc.scalar.activation(dst, tp[:, 0:w], _ACT.Copy,
                                                 scale=0.5)
                        else:
                            nc.vector.tensor_scalar_mul(dst, tp[:, 0:w], 0.5)

        # phase-2 shared resources (vocab projection), usable both inside the
        # recurrence (idle-PE interleave) and in the tail loop
        ones1 = pers.tile([1, 128], BF, tag="ones1")
        nc.vector.memset(ones1[:], 1.0)
        outb_sb = pers.tile([1, V], BF, tag="outb_sb")
        nc.sync.dma_start(outb_sb[:], outb_d[:])

        # G_emb computed on device (ships emb.T [128, 1024] instead of the
        # 4x larger gate-space gemb): G_emb = emb @ G_W.T + G_bias, stored
        # bf16 as two row-halves gemb_sb[:, 2048h + g] with psum row
        # r = 4*t_rel + b, t = 32h + t_rel.
        embT = pers.tile([128, KCH * 2 * 128], BF, tag="embT")
        gwT = pers.tile([128, KCH * G], BF, tag="gwT")
        gemb_sb = pers.tile([128, 2 * G], BF, tag="gemb_sb")
        gbias_sb = pers.tile([1, G], BF, tag="gbias_sb")
        nc.sync.dma_start(embT[:], embT_d[:])
        nc.sync.dma_start(gwT[:], gwT_d[:])
        nc.sync.dma_start(gbias_sb[:], gbias_d[:])
        with tc.tile_pool(name="ps_ge", bufs=1, space="PSUM") as ps_ge:
            for h in range(2):
                gp = ps_ge.tile([128, G], F32, tag="gep")
                for n in range(4):
                    gsl = slice(512 * n, 512 * n + 512)
                    for k in range(KCH):
                        mm(gp[:, gsl],
                           embT[:, 256 * k + 128 * h: 256 * k + 128 * h + 128],
                           gwT[:, G * k + 512 * n: G * k + 512 * n + 512],
                           start=(k == 0), stop=False)
                    mm(gp[:, gsl], ones1[0:1, 0:128], gbias_sb[0:1, gsl],
                       start=False, stop=True)
                if h == 0:
                    nc.scalar.copy(gemb_sb[:, 0:G], gp[:])
                else:
                    nc.vector.tensor_copy(gemb_sb[:, G:2 * G], gp[:])
        wsb = ctx.enter_context(tc.tile_pool(name="wsb", bufs=12))
        osb = ctx.enter_context(tc.tile_pool(name="osb", bufs=4))
        qsb = ctx.enter_context(tc.tile_pool(name="qsb", bufs=4))
        nvoc = NVOC
        mch = MCH

        def emit_p2(m, n, ps_pool, eng_flip):
            # deprioritize against the recurrence chain for engine contention
            tc.cur_priority += 50000
            _emit_p2_body(m, n, ps_pool, eng_flip)
            tc.cur_priority -= 50000

        def _emit_p2_body(m, n, ps_pool, eng_flip):
            nw = min(512, V - 512 * n)
            mr = min(128, BC * seq - 128 * m)
            wts = []
            for k in range(KCH):
                wt = wsb.tile([128, 512], BF, tag="wt")
                nc.sync.dma_start(
                    wt[:, 0:nw],
                    outWT_d[128 * k: 128 * k + 128, 512 * n: 512 * n + nw])
                wts.append(wt)
            ps = ps_pool.tile([128, 512], F32, tag="po")
            for k in range(KCH):
                mm(ps[0:mr, 0:nw],
                   HT[:, ht_cols * k + 4 + 128 * m:
                      ht_cols * k + 4 + 128 * m + mr],
                   wts[k][:, 0:nw],
                   start=(k == 0), stop=False)
            mm(ps[0:mr, 0:nw], ones1[0:1, 0:mr],
               outb_sb[0:1, 512 * n: 512 * n + nw],
               start=False, stop=True)
            # int8-quantize per output row: q = round(x * 127/absmax(row)),
            # scale[row] = absmax/127 shipped alongside (HW convert is
            # round-to-nearest with saturation).
            mx = qsb.tile([128, 1], F32, tag="qmx")
            nc.vector.tensor_reduce(mx[0:mr], ps[0:mr, 0:nw],
                                    axis=mybir.AxisListType.X,
                                    op=mybir.AluOpType.max,
                                    apply_absolute_value=True)
            rq = qsb.tile([128, 1], F32, tag="qrq")
            nc.vector.reciprocal(rq[0:mr], mx[0:mr])
            nc.vector.tensor_scalar_mul(rq[0:mr], rq[0:mr], 127.0)
            nc.vector.tensor_scalar_mul(
                scales[0:mr, nvoc * m + n: nvoc * m + n + 1],
                mx[0:mr], 1.0 / 127.0)
            ob = osb.tile([128, 512], I8, tag="ob")
            if eng_flip:
                nc.scalar.activation(ob[0:mr, 0:nw], ps[0:mr, 0:nw],
                                     _ACT.Copy, scale=rq[0:mr])
            else:
                nc.vector.tensor_scalar_mul(ob[0:mr, 0:nw], ps[0:mr, 0:nw],
                                            rq[0:mr])
            dst = out_d[:, 32 * m: 32 * m + mr // 4, 512 * n: 512 * n + nw]
            nc.sync.dma_start(dst.rearrange("b t v -> t b v"), ob[0:mr, 0:nw])

        # ---------------- recurrence ----------------
        with ExitStack() as rctx:
            sb = rctx.enter_context(tc.tile_pool(name="sb", bufs=2))
            ps_sc = rctx.enter_context(
                tc.tile_pool(name="ps_sc", bufs=1, space="PSUM"))
            ps_tp = rctx.enter_context(
                tc.tile_pool(name="ps_tp", bufs=1, space="PSUM"))
            ps_g = rctx.enter_context(
                tc.tile_pool(name="ps_g", bufs=1, space="PSUM"))
            ps_oi = rctx.enter_context(
                tc.tile_pool(name="ps_oi", bufs=1, space="PSUM"))
            p2_done = 0  # m=0 vocab chunks emitted inside the recurrence

            # scores psum: batch b's scores live in row 32*b (col-group
            # tile_position); untouched rows stay 0 from this one memset.
            psum_s = ps_sc.tile([128, 512], F32, tag="ps_s")
            nc.vector.memset(psum_s[:], 0.0)

            for t in range(seq):
                hc = 4 * t

                # scores row for batch b at partition 32b:
                # psum_s[32b, n] = <h_b, F[b,n,:]>
                for b in range(BC):
                    for k in range(KCH):
                        mm(psum_s[32 * b: 32 * b + 1, 0:NREG],
                           hT2[:, 8 * k + 2 * b: 8 * k + 2 * b + 1],
                           fT[:, BC * NREG * k + NREG * b:
                              BC * NREG * k + NREG * (b + 1)],
                           start=(k == 0), stop=(k == KCH - 1),
                           tile_position=(0, 32 * b))

                # gates part 1: h @ Whh.T + G_emb  (PE work hiding softmax).
                # G_emb rows for step t come from gemb_sb via an eye128
                # column-select (rows 4*(t%32)..+4 of half t//32).
                gps = ps_g.tile([BC, G], F32, tag="gps")
                tr4 = 4 * (t % 32)
                gh = G * (t // 32)
                for n in range(4):
                    gsl = slice(512 * n, 512 * n + 512)
                    for k in range(KCH):
                        mm(gps[:, gsl],
                           HT[:, ht_cols * k + hc: ht_cols * k + hc + 4],
                           whhT[:, G * k + 512 * n: G * k + 512 * n + 512],
                           start=(k == 0), stop=False)
                    mm(gps[:, gsl], eye128[:, tr4: tr4 + 4],
                       gemb_sb[:, gh + 512 * n: gh + 512 * n + 512],
                       start=False, stop=False)

                # softmax along the free dim, rows {0,32,64,96} meaningful
                mx = sb.tile([128, 1], F32, tag="mx")
                nc.vector.reduce_max(mx[:], psum_s[:, 0:NREG],
                                     axis=mybir.AxisListType.X)
                nmx = sb.tile([128, 1], F32, tag="nmx")
                nc.vector.tensor_scalar_mul(nmx[:], mx[:], -1.0)
                ssum = sb.tile([128, 1], F32, tag="ssum")
                nc.scalar.activation(attn_bf[:, 0:NREG], psum_s[:, 0:NREG], _ACT.Exp,
                                     bias=nmx[:], scale=1.0, accum_out=ssum[:])
                rinv = sb.tile([128, 1], F32, tag="rinv")
                nc.vector.reciprocal(rinv[:], ssum[:])
                nc.vector.tensor_scalar_mul(attn_bf[:, 0:NREG],
                                            attn_bf[:, 0:NREG], rinv[:])

                # attn.T via row-wise PE transposes -> block-diag scatter
                atp = ps_tp.tile([128, 4 * BC], BF, tag="tpb")
                for b in range(BC):
                    for k2 in range(2):
                        c2 = 2 * b + k2
                        mm(atp[:, 2 * c2: 2 * c2 + 1],
                           attn_bf[32 * b: 32 * b + 1,
                                   128 * k2: 128 * (k2 + 1)],
                           onescol[32 * b: 32 * b + 1, 0:1],
                           is_transpose=True, tile_position=(32 * b, 0))
                nc.vector.memset(BD[:], 0.0)
                # dst col 4*(2b+k2)+b = 9b+4k2, src col 2*(2b+k2) = 4b+2k2:
                # both affine in (b, k2) -> a single strided-AP copy
                bd_dst = bass.AP(BD.tensor, BD.offset,
                                 [BD.ap[0], [9, BC], [4, 2]])
                bd_src = bass.AP(atp.tensor, atp.offset,
                                 [atp.ap[0], [4, BC], [2, 2]])
                nc.scalar.copy(bd_dst, bd_src)

                # context transposed: ctxT[e, b]
                cps = ps_tp.tile([128, 4 * KCH], F32, tag="cps")
                for m in range(KCH):
                    for c2 in range(2 * BC):
                        mm(cps[:, 4 * m: 4 * m + 4],
                           fP[:, 512 * c2 + 128 * m: 512 * c2 + 128 * m + 128],
                           BD[:, 4 * c2: 4 * c2 + 4],
                           start=(c2 == 0), stop=(c2 == 2 * BC - 1))
                nc.scalar.copy(ctxT[:], cps[:])

                # gates part 2: ctx @ W_cg.T
                for n in range(4):
                    gsl = slice(512 * n, 512 * n + 512)
                    for k in range(KCH):
                        mm(gps[:, gsl],
                           ctxT[:, 4 * k: 4 * k + 4],
                           wcgT[:, G * k + 512 * n: G * k + 512 * n + 512],
                           start=False, stop=(k == KCH - 1))

                # vocab-projection chunks for rows t<32 interleave into the
                # idle PE window left by the elementwise chain (also keeps
                # the PE p-state warm)
                import os as _os
                if seq == SEQ and t >= 33 and not _os.environ.get("K_NO_P2"):
                    quota = min(nvoc, 2 * (t - 32))
                    while p2_done < quota:
                        emit_p2(0, p2_done, ps_oi, p2_done % 2 == 0)
                        p2_done += 1

                # LSTM cell via tanh-only activations (one ACT table set).
                # sigma(x) = 0.5(1+tanh(x/2)); h is stored as 2h with the
                # 0.5 factors folded into fT/whhT/outWT/h0T on the host, so
                # each sigma-multiply fuses into one scalar_tensor_tensor:
                #   u0 = (1+th_f)*c = 2*sig(f)*c
                #   u1 = (1+th_i)*tg = 2*sig(i)*tanh(g)
                #   v = u0+u1 = 2*c2;  c <- 0.5v;  tanh(c2) = Tanh(0.5*v)
                #   h2x2 = (1+th_o)*tanh(c2) = 2*h2
                mlop = mybir.AluOpType.mult
                adop = mybir.AluOpType.add
                thif = sb.tile([BC, 1024], F32, tag="thif")
                nc.scalar.activation(thif[:], gps[:, 0:1024], _ACT.Tanh,
                                     scale=0.5)
                tg = sb.tile([BC, 512], F32, tag="tg")
                nc.scalar.activation(tg[:], gps[:, 1024:1536], _ACT.Tanh)
                tho = sb.tile([BC, 512], F32, tag="tho")
                nc.scalar.activation(tho[:], gps[:, 1536:2048], _ACT.Tanh,
                                     scale=0.5)
                u0 = sb.tile([BC, 512], F32, tag="u0")
                nc.vector.scalar_tensor_tensor(u0[:], thif[:, 512:1024], 1.0,
                                               cst[:], adop, mlop)
                u1 = sb.tile([BC, 512], F32, tag="u1")
                nc.vector.scalar_tensor_tensor(u1[:], thif[:, 0:512], 1.0,
                                               tg[:], adop, mlop)
                v2c = sb.tile([BC, 512], F32, tag="v2c")
                nc.vector.tensor_add(v2c[:], u0[:], u1[:])
                tc2 = sb.tile([BC, 512], F32, tag="tc2")
                nc.scalar.activation(tc2[:], v2c[:], _ACT.Tanh, scale=0.5)
                nc.vector.tensor_scalar_mul(cst[:], v2c[:], 0.5)
                h2 = sb.tile([BC, 512], BF, tag="h2")
                nc.vector.scalar_tensor_tensor(h2[:], tho[:], 1.0, tc2[:],
                                               adop, mlop)

                # h2.T -> HT col group t+1
                hps = ps_tp.tile([128, 4 * KCH], BF, tag="tpb")
                for m in range(KCH):
                    nc.tensor.transpose(hps[:, 4 * m: 4 * m + 4],
                                        h2[0:4, 128 * m: 128 * m + 128],
                                        eye4[:])
                ht_dst = bass.AP(HT.tensor, HT.offset + 4 * (t + 1),
                                 [HT.ap[0], [ht_cols, KCH], [1, 4]])
                nc.scalar.copy(ht_dst, hps[:].rearrange(
                    "p (m c) -> p m c", m=KCH))
                h2_dst = bass.AP(hT2.tensor, hT2.offset,
                                 [hT2.ap[0], [8, KCH], [2, 4]])
                nc.vector.tensor_copy(h2_dst, hps[:].rearrange(
                    "p (m c) -> p m c", m=KCH))

        # ------- phase 2 tail: remaining vocab-projection chunks -----------
        import os
        if os.environ.get("K_SKIP_P2") or os.environ.get("K_NO_P2"):
            return
        with ExitStack() as ctx2:
            ps_o2 = ctx2.enter_context(
                tc.tile_pool(name="ps_o2", bufs=4, space="PSUM"))
            rest = []
            if seq == SEQ:
                try:
                    rest += [(0, n) for n in range(p2_done, nvoc)]
                except NameError:
                    rest += [(0, n) for n in range(nvoc)]
                rest += [(m, n) for m in range(1, mch) for n in range(nvoc)]
            else:
                rest += [(m, n) for m in range(mch) for n in range(nvoc)]
            for i, (m, n) in enumerate(rest):
                emit_p2(m, n, ps_o2, i % 2 == 0)
        nc.sync.dma_start(scal_d[:], scales[:])


def host_fold_weights(inputs):
    """Fold the weight tensors into the const arrays baked into the NEFF."""
    f32 = np.float32
    reshape_W = np.asarray(inputs["reshape_W"], f32)
    reshape_b = np.asarray(inputs["reshape_b"], f32)
    Wih = np.asarray(inputs["lstm_Wih"], f32)
    Whh = np.asarray(inputs["lstm_Whh"], f32)
    bih = np.asarray(inputs["lstm_bih"], f32)
    bhh = np.asarray(inputs["lstm_bhh"], f32)
    out_W = np.asarray(inputs["out_W"], f32)
    out_b = np.asarray(inputs["out_b"], f32)

    R1, R2 = reshape_W[:, :E], reshape_W[:, E:]
    W_cg = Wih @ R1
    G_W = Wih @ R2
    G_bias = reshape_b @ Wih.T + bih + bhh

    def kmajor(x):   # [512, cols] -> [128, 4*cols], col = cols*k + c
        c = x.shape[1]
        return np.ascontiguousarray(
            x.reshape(KCH, 128, c).transpose(1, 0, 2).reshape(128, KCH * c))

    return {
        "wcgT": kmajor(W_cg.T).astype(BF16),
        "whhT": kmajor(0.5 * Whh.T).astype(BF16),
        "outWT": np.ascontiguousarray(0.5 * out_W.T).astype(BF16),
        "outb": out_b.reshape(1, V).astype(BF16),
        "eye4": np.eye(BC, dtype=BF16),
        "eye128": np.eye(128, dtype=BF16),
        "gwT": kmajor(G_W.T).astype(BF16),
        "gbias": G_bias.reshape(1, G).astype(BF16),
    }


def host_prep_data(inputs, seq=SEQ):
    """Per-call data inputs -> the 8 per-core in_maps (weights excluded)."""
    f32 = np.float32
    features = np.asarray(inputs["features"], f32)
    captions = np.asarray(inputs["captions"])
    embed_W = np.asarray(inputs["embed_W"], f32)
    init_h_W = np.asarray(inputs["init_h_W"], f32)
    init_h_b = np.asarray(inputs["init_h_b"], f32)
    init_c_W = np.asarray(inputs["init_c_W"], f32)
    init_c_b = np.asarray(inputs["init_c_b"], f32)

    emb = embed_W[captions] * np.sqrt(f32(E))           # [B, S, E]
    fmean = features.mean(axis=1)
    h0 = fmean @ init_h_W.T + init_h_b
    c0 = fmean @ init_c_W.T + init_c_b

    in_maps = []
    for c in range(NCORES):
        bs = slice(BC * c, BC * (c + 1))
        Fc = features[bs]
        fP0 = Fc[:, :128].transpose(1, 0, 2).reshape(128, BC * E)
        fP1 = Fc[:, 128:NREG].transpose(1, 0, 2).reshape(NREG - 128, BC * E)
        h0T = (2.0 * h0[bs].T.reshape(KCH, 128, BC)
               .transpose(1, 0, 2).reshape(128, KCH * BC))
        # embT col = 256k + 128h + 4*t_rel + b  (t = 32h + t_rel)
        embT = (emb[bs, :seq].transpose(2, 1, 0)        # [e, t, b]
                .reshape(KCH, 128, 2, 32, BC)
                .transpose(1, 0, 2, 3, 4).reshape(128, KCH * 2 * 128))
        in_maps.append({
            "fP0": np.ascontiguousarray(fP0).astype(BF16),
            "fP1": np.ascontiguousarray(fP1).astype(BF16),
            "h0T": np.ascontiguousarray(h0T).astype(BF16),
            "c0": np.ascontiguousarray(c0[bs]),
            "embT": np.ascontiguousarray(embT).astype(BF16),
        })
    return in_maps


def assemble_output(outs):
    """Dequantize int8+scales global outputs -> [B, SEQ, V] float32.

    Output row r of row-chunk m maps to (t = 32m + r//4, b_local = r%4);
    scales column index is m*NVOC + n for vocab chunk n.
    """
    O = outs["out"]          # [B, SEQ, V] int8   (B = NCORES*BC)
    S = outs["scales"]       # [NCORES*128, MCH*NVOC] f32
    Sb = (S.reshape(NCORES, 128, MCH, NVOC)
          .reshape(NCORES, 32, BC, MCH, NVOC)
          .transpose(0, 2, 3, 1, 4)        # [core, b, m, t_rel, n]
          .reshape(B, SEQ, NVOC))
    out = np.empty((B, SEQ, V), np.float32)
    nfull = V // 512                       # full 512-wide chunks
    vf = nfull * 512
    out[:, :, :vf] = (O[:, :, :vf].reshape(B, SEQ, nfull, 512)
                      * Sb[:, :, :nfull, None]).reshape(B, SEQ, vf)
    out[:, :, vf:] = O[:, :, vf:] * Sb[:, :, nfull, None]
    return out


# ---------------------------------------------------------------------------
# Cached PJRT runner: jit the bass program ONCE per weight-set; later calls
# only pay input transfer + execute + output fetch.

class _Bundle:
    def __init__(self, nc):
        import jax
        from jax.experimental.shard_map import shard_map
        from jax.sharding import Mesh, PartitionSpec

        _bass2jax.install_neuronx_cc_hook()
        self.nc = nc
        partition_name = (nc.partition_id_tensor.name
                          if nc.partition_id_tensor else None)
        in_names, out_names, out_avals = [], [], []
        for alloc in nc.m.functions[0].allocations:
            if not isinstance(alloc, mybir.MemoryLocationSet):
                continue
            name = alloc.memorylocations[0].name
            if alloc.kind == "ExternalInput":
                if name != partition_name:
                    in_names.append(name)
            elif alloc.kind == "ExternalOutput":
                out_names.append(name)
                out_avals.append(jax.core.ShapedArray(
                    tuple(alloc.tensor_shape), mybir.dt.np(alloc.dtype)))
        self.in_names, self.out_names = in_names, out_names
        self.out_avals = out_avals
        bind_in_names = list(in_names)
        if partition_name is not None:
            bind_in_names.append(partition_name)

        def _body(*args):
            operands = list(args)
            if partition_name is not None:
                operands.append(_bass2jax.partition_id_tensor())
            outs = _bass2jax._bass_exec_p.bind(
                *operands,
                out_avals=tuple(out_avals),
                in_names=tuple(bind_in_names),
                out_names=tuple(out_names),
                lowering_input_output_aliases=(),
                sim_require_finite=True,
                sim_require_nnan=True,
                nc=nc,
            )
            return tuple(outs)

        devices = jax.devices()[:NCORES]
        assert len(devices) == NCORES
        mesh = Mesh(np.asarray(devices), ("core",))
        P = PartitionSpec
        self.jitfn = jax.jit(shard_map(
            _body, mesh=mesh,
            in_specs=(P("core"),) * len(in_names),
            out_specs=(P("core"),) * len(out_names),
            check_rep=False))

    def run(self, in_maps):
        """in_maps (host numpy, one dict per core) -> global outputs on host
        (dict name -> [NCORES*dim0, ...])."""
        import jax
        concat_in = [
            np.concatenate([m[name] for m in in_maps], axis=0)
            for name in self.in_names
        ]
        out_arrs = self.jitfn(*concat_in)
        # fetch all output shards concurrently (the axon link serializes a
        # single np.asarray shard-by-shard at ~2/3 of its aggregate rate)
        shard_data, meta = [], []
        for name, arr in zip(self.out_names, out_arrs):
            for s in arr.addressable_shards:
                shard_data.append(s.data)
                meta.append((name, s.index))
        vals = jax.device_get(shard_data)
        result = {}
        for i, (name, arr) in enumerate(zip(self.out_names, out_arrs)):
            g = np.empty(
                (NCORES * self.out_avals[i].shape[0],
                 *self.out_avals[i].shape[1:]),
                self.out_avals[i].dtype)
            result[name] = g
        for (name, idx), v in zip(meta, vals):
            result[name][idx] = v
        return result


_bundle_cache = {}


def get_bundle(inputs):
    key = hashlib.blake2b(
        b"".join(np.ascontiguousarray(np.asarray(inputs[k], np.float32))
                 .tobytes()
                 for k in ("reshape_W", "reshape_b", "lstm_Wih", "lstm_Whh",
                           "lstm_bih", "lstm_bhh", "out_W", "out_b")),
        digest_size=16).hexdigest()
    if key not in _bundle_cache:
        consts = host_fold_weights(inputs)
        nc = build_program(consts, SEQ)
        _bundle_cache[key] = _Bundle(nc)
    return _bundle_cache[key]


def kernel(**inputs) -> np.ndarray:
    bundle = get_bundle(inputs)
    in_maps = host_prep_data(inputs, SEQ)
    outs = bundle.run(in_maps)
    return assemble_output(outs)                 # [32, SEQ, V] f32


if __name__ == "__main__":
    import reference as refmod
    inputs = {k: np.asarray(v) for k, v in refmod.setup_inputs().items()}
    expected = np.asarray(refmod.reference(**inputs))
    got = kernel(**inputs)
    err = np.abs(got - expected).max() / np.abs(expected).max()
    l2 = np.linalg.norm((got - expected).ravel()) / np.linalg.norm(expected.ravel())
    print(f"Relative error: {err:.3e} (l2 {l2:.3e})")


# revision 41
# speedup vs baseline: 1.0878x; 1.0286x over previous
"""DecoderLSTM (attention + LSTM + vocab projection) on 8 Trainium2 NeuronCores.

Strategy (data-parallel over batch, no collectives):
  - Each of the 8 cores owns 4 of the 32 batch elements and runs the full
    64-step attention-LSTM recurrence for them in bf16 (fp32 cell state),
    storing h_t transposed in SBUF.
  - The vocab projection (90% of FLOPs) is hoisted out of the recurrence:
    one dense [256,512]@[512,32000] matmul per core, streaming out_W.T
    from HBM, partially interleaved into the recurrence's idle PE windows.
  - Algebraic folds done on host (numpy): the embedding gather, h0/c0 init,
    reshape_W folded into the LSTM input weights (W_cg = Wih @ R1), and the
    per-step embedding contribution G_emb[t] = emb_t @ (Wih R2).T + biases.

Wall-clock engineering (the axon tunnel runs at ~40 MB/s, so bytes moved
per call dominate end-to-end latency):
  - All weight tensors (wcgT/whhT/outWT/outb/gwT/gbias + identities) are
    baked into the NEFF as Const tensors (inline_tensor): they ride in the
    executable at load time and are NOT re-transferred per call. kernel()
    re-specializes (retrace + recompile) if the weight values change.
  - The PJRT executable is jitted ONCE per weight-set and cached; calls
    after the first skip trace/lower/compile entirely.
  - No zero output buffers are shipped (the kernel writes every output
    element, so uninitialized XLA output buffers are fine).
  - Per-call H2D is only the data-dependent inputs (~8.7 MB total):
    unpadded features (fP0/fP1), transposed token embeddings (embT), and
    h0/c0. fT (the transposed features layout) and G_emb (the gate-space
    embedding contribution) are derived on device.
  - The logits cross the tunnel int8-quantized with per-row scales
    (absmax/127, hardware round-to-nearest), ~66 MB instead of 262 MB
    f32; the host dequantizes. Output shards are fetched concurrently
    (jax.device_get on the shard list) — a lone np.asarray walks shards
    serially at ~2/3 the rate.

Numerics: bf16 matmuls with fp32 accumulation + int8 output quantization
-> rel err ~9e-3 vs fp32 ref (gate: 2e-2). All ScalarE activations stay
inside the single "exp_and_others" table set (exp, tanh, copy); sigmoid
is computed as tanh via sigma(x) = 0.5(1+tanh(x/2)) with the 0.5 factors
folded into the weights.
"""

import hashlib
from contextlib import ExitStack

import numpy as np
import ml_dtypes

import bass_rust
import concourse.bass as bass
import concourse.tile as tile
import concourse.mybir as mybir

BF16 = ml_dtypes.bfloat16
F32 = mybir.dt.float32
BF = mybir.dt.bfloat16
I8 = mybir.dt.int8

NCORES = 8
B = 32            # total batch
BC = 4            # batch per core
NREG = 196        # attention regions
NPAD = 256        # padded regions (2 chunks of 128 per batch element)
E = 512           # embed dim == hidden dim
G = 2048          # gate dim (4*H)
SEQ = 64
V = 32000
KCH = E // 128    # 4 k-chunks of the hidden dim
NVOC = (V + 511) // 512   # 63 vocab chunks of 512
VP = NVOC * 512           # 32256, int8 output padded to chunk multiple
MCH = (BC * SEQ + 127) // 128  # 2 row-chunks of the (t, b) dim

_ACT = mybir.ActivationFunctionType

# ---------------------------------------------------------------------------
# Workaround for a walrus codegen limit: an InstDrain may carry only one sync
# wait, but TileContext._drain_and_barrier attaches every outstanding proc's
# wait to one tail drain. Split the waits across a chain of drains.


def _split_drain_and_barrier(self, tick_clock, wait_clock):
    nc = self.nc
    drain_inst = nc.sync.drain()
    wait_clock.add_sem_waits(
        drain_inst.ins, bass_rust.ScopedClock({None: tick_clock.global_clock})
    )
    si = drain_inst.ins.sync_info
    if si is not None and si.on_wait is not None and len(si.on_wait) > 1:
        waits = list(si.on_wait)
        si.on_wait = waits[:1]
        for w in waits[1:]:
            d2 = nc.sync.drain()
            d2.ins.sync_info = bass_rust.SyncInfo(on_wait=[w], on_update=[])
    nc.all_engine_barrier()
    assert self.sems is not None
    popped = nc._tile_sem_poison_stack.pop()
    assert popped is self._sem_poison
    nc.clear_and_free_semaphores(list(self.sems.allocated().values()))
    nc.all_engine_barrier()


tile.TileContext._drain_and_barrier = _split_drain_and_barrier


# This walrus build rejects ANY instruction carrying more than one sync wait
# ("Too many sync wait commands"), while Tile freely attaches one wait per
# producer. General fix: post-process the BIR JSON, hoisting excess waits
# onto single-wait Drain instructions inserted just before the offender on
# the same engine.
def _split_multiwait_bir(bir_bytes):
    import orjson
    d = orjson.loads(bir_bytes)
    ctr = 0
    for f in d["functions"]:
        for bb in f["blocks"]:
            insts = bb.get("instructions")
            if not insts:
                continue
            out = []
            changed = False
            for inst in insts:
                si = inst.get("sync_info")
                waits = (si or {}).get("on_wait") or []
                cap = 2 if inst.get("opcode") == "EventSemaphore" else 1
                if len(waits) > cap:
                    changed = True
                    for w in waits[:-cap]:
                        ctr += 1
                        out.append({
                            "engine": inst["engine"],
                            "ins": [],
                            "name": f"I-mwsplit-{ctr}",
                            "opcode": "Drain",
                            "outs": [],
                            "sync_info": {"on_update": [], "on_wait": [w]},
                        })
                    si["on_wait"] = waits[-cap:]
                out.append(inst)
            if changed:
                bb["instructions"] = out
    return orjson.dumps(d)


from concourse import bass_utils  # noqa: E402
from concourse import bass2jax as _bass2jax  # noqa: E402

_orig_compile_bir_kernel = bass_utils.compile_bir_kernel


def _patched_compile_bir_kernel(bir_json, tmpdir, neff_name="file.neff"):
    return _orig_compile_bir_kernel(_split_multiwait_bir(bir_json), tmpdir,
                                    neff_name)


bass_utils.compile_bir_kernel = _patched_compile_bir_kernel
_bass2jax.compile_bir_kernel = _patched_compile_bir_kernel
# ---------------------------------------------------------------------------


def build_program(consts, seq=SEQ):
    """Trace the per-core Tile program. Weight arrays in `consts` are baked
    into the NEFF as Const tensors. Returns the Bass module."""
    nc = bass.Bass("TRN2", target_bir_lowering=False, debug=False,
                   num_devices=NCORES)

    dt = nc.dram_tensor
    fP0_d = dt("fP0", [128, BC * E], BF, kind="ExternalInput")
    fP1_d = dt("fP1", [NREG - 128, BC * E], BF, kind="ExternalInput")
    h0T_d = dt("h0T", [128, 4 * KCH], BF, kind="ExternalInput")
    c0_d = dt("c0", [BC, E], F32, kind="ExternalInput")
    embT_d = dt("embT", [128, KCH * 2 * 128], BF, kind="ExternalInput")
    wcgT_d = nc.inline_tensor(consts["wcgT"], "wcgT")
    whhT_d = nc.inline_tensor(consts["whhT"], "whhT")
    outWT_d = nc.inline_tensor(consts["outWT"], "outWT")
    outb_d = nc.inline_tensor(consts["outb"], "outb")
    eye4_d = nc.inline_tensor(consts["eye4"], "eye4")
    eye128_d = nc.inline_tensor(consts["eye128"], "eye128")
    gwT_d = nc.inline_tensor(consts["gwT"], "gwT")
    gbias_d = nc.inline_tensor(consts["gbias"], "gbias")
    out_d = dt("out", [BC, seq, V], I8, kind="ExternalOutput")
    scal_d = dt("scales", [128, MCH * NVOC], F32, kind="ExternalOutput")

    with tile.TileContext(nc) as tc:
        _trace(nc, tc, seq,
               fP0_d.ap(), fP1_d.ap(), h0T_d.ap(), c0_d.ap(), embT_d.ap(),
               wcgT_d.ap(), whhT_d.ap(), outWT_d.ap(), outb_d.ap(),
               eye4_d.ap(), eye128_d.ap(), gwT_d.ap(), gbias_d.ap(),
               out_d.ap(), scal_d.ap())
    return nc


def _trace(nc, tc, seq, fP0_d, fP1_d, h0T_d, c0_d, embT_d, wcgT_d, whhT_d,
           outWT_d, outb_d, eye4_d, eye128_d, gwT_d, gbias_d, out_d, scal_d):
    ht_cols = 4 * (seq + 1)
    mm = nc.tensor.matmul

    with ExitStack() as ctx:
        # ---------------- persistent SBUF (spans both phases) --------------
        pers = ctx.enter_context(tc.tile_pool(name="pers", bufs=1))
        fT = pers.tile([128, KCH * BC * NREG], BF, tag="fT")
        fP = pers.tile([128, 2 * BC * E], BF, tag="fP")
        wcgT = pers.tile([128, KCH * G], BF, tag="wcgT")
        whhT = pers.tile([128, KCH * G], BF, tag="whhT")
        HT = pers.tile([128, KCH * ht_cols], BF, tag="HT")  # col=ht_cols*k+4t+b
        cst = pers.tile([BC, E], F32, tag="cst")
        eye4 = pers.tile([BC, BC], BF, tag="eye4")
        eye128 = pers.tile([128, 128], BF, tag="eye128")
        onescol = pers.tile([128, 1], BF, tag="onescol")
        # current h, transposed, with stride-2 columns (col = 8k + 2b) so each
        # single-column matmul lhsT is 4-byte aligned in bf16
        hT2 = pers.tile([128, 8 * KCH], BF, tag="hT2")
        attn_bf = pers.tile([128, NPAD], BF, tag="attn_bf")
        BD = pers.tile([128, 4 * 2 * BC], BF, tag="BD")
        ctxT = pers.tile([128, 4 * KCH], BF, tag="ctxT")
        scales = pers.tile([128, MCH * NVOC], F32, tag="scales")

        # fP is shipped without the region padding: rc=0 rows fully, rc=1
        # only the 68 valid rows. Pad rows are zeroed (attention weights for
        # pad regions are zero, but NaN garbage would poison 0*x in the PE).
        fP_4d = fP[:].rearrange("p (b rc e) -> p b rc e", b=BC, rc=2)
        nc.sync.dma_start(
            fP_4d[:, :, 0, :],
            fP0_d[:].rearrange("p (b e) -> p b e", b=BC))
        # zero the pad rows first (DVE partition base must be 32-aligned,
        # so clear [64:128] and let the fP1 DMA overwrite rows 64..67)
        for b in range(BC):
            nc.vector.memset(
                fP[64:128, 1024 * b + 512: 1024 * b + 1024], 0.0)
        nc.sync.dma_start(
            fP_4d[0:NREG - 128, :, 1, :],
            fP1_d[:].rearrange("p (b e) -> p b e", b=BC))
        nc.sync.dma_start(wcgT[:], wcgT_d[:])
        nc.sync.dma_start(whhT[:], whhT_d[:])
        nc.sync.dma_start(cst[:], c0_d[:])
        nc.sync.dma_start(eye4[:], eye4_d[:])
        nc.sync.dma_start(eye128[:], eye128_d[:])
        nc.sync.dma_start(
            HT[:].rearrange("p (k c) -> p k c", k=KCH)[:, :, 0:4],
            h0T_d[:].rearrange("p (k c) -> p k c", k=KCH))
        nc.sync.dma_start(
            hT2[:].rearrange("p (k b two) -> p k b two", k=KCH, two=2)
            [:, :, :, 0:1],
            h0T_d[:].rearrange("p (k b one) -> p k b one", k=KCH, one=1))
        nc.vector.memset(onescol[:], 1.0)
        nc.vector.memset(attn_bf[:, NREG:NPAD], 0.0)

        # fT derived on device from fP (saves shipping the second features
        # layout over the slow host link): fT[:, NREG*(BC*k+b)+128*rc ...] =
        # 0.5 * transpose(fP block for (b, rc, k)).  The 0.5 is the sigma
        # x2-h folding factor (see the LSTM cell comment below).
        with tc.tile_pool(name="ps_ft", bufs=2, space="PSUM") as ps_ft:
            for k in range(KCH):
                for b in range(BC):
                    for rc in range(2):
                        w = 128 if rc == 0 else NREG - 128
                        tp = ps_ft.tile([128, 128], BF, tag="ftp")
                        nc.tensor.transpose(
                            tp[:],
                            fP[:, 1024 * b + 512 * rc + 128 * k:
                               1024 * b + 512 * rc + 128 * k + 128],
                            eye128[:])
                        dst = fT[:, NREG * (BC * k + b) + 128 * rc:
                                 NREG * (BC * k + b) + 128 * rc + w]
                        if (b + rc) % 2 == 0:
                            nc.scalar.activation(dst, tp[:, 0:w], _ACT.Copy,
                                                 scale=0.5)
                        else:
                            nc.vector.tensor_scalar_mul(dst, tp[:, 0:w], 0.5)

        # phase-2 shared resources (vocab projection), usable both inside the
        # recurrence (idle-PE interleave) and in the tail loop
        ones1 = pers.tile([1, 128], BF, tag="ones1")
        nc.vector.memset(ones1[:], 1.0)
        outb_sb = pers.tile([1, V], BF, tag="outb_sb")
        nc.sync.dma_start(outb_sb[:], outb_d[:])

        # G_emb computed on device (ships emb.T [128, 1024] instead of the
        # 4x larger gate-space gemb): G_emb = emb @ G_W.T + G_bias, stored
        # bf16 as two row-halves gemb_sb[:, 2048h + g] with psum row
        # r = 4*t_rel + b, t = 32h + t_rel.
        embT = pers.tile([128, KCH * 2 * 128], BF, tag="embT")
        gwT = pers.tile([128, KCH * G], BF, tag="gwT")
        gemb_sb = pers.tile([128, 2 * G], BF, tag="gemb_sb")
        gbias_sb = pers.tile([1, G], BF, tag="gbias_sb")
        nc.sync.dma_start(embT[:], embT_d[:])
        nc.sync.dma_start(gwT[:], gwT_d[:])
        nc.sync.dma_start(gbias_sb[:], gbias_d[:])
        with tc.tile_pool(name="ps_ge", bufs=1, space="PSUM") as ps_ge:
            for h in range(2):
                gp = ps_ge.tile([128, G], F32, tag="gep")
                for n in range(4):
                    gsl = slice(512 * n, 512 * n + 512)
                    for k in range(KCH):
                        mm(gp[:, gsl],
                           embT[:, 256 * k + 128 * h: 256 * k + 128 * h + 128],
                           gwT[:, G * k + 512 * n: G * k + 512 * n + 512],
                           start=(k == 0), stop=False)
                    mm(gp[:, gsl], ones1[0:1, 0:128], gbias_sb[0:1, gsl],
                       start=False, stop=True)
                if h == 0:
                    nc.scalar.copy(gemb_sb[:, 0:G], gp[:])
                else:
                    nc.vector.tensor_copy(gemb_sb[:, G:2 * G], gp[:])
        wsb = ctx.enter_context(tc.tile_pool(name="wsb", bufs=12))
        osb = ctx.enter_context(tc.tile_pool(name="osb", bufs=4))
        qsb = ctx.enter_context(tc.tile_pool(name="qsb", bufs=4))
        nvoc = NVOC
        mch = MCH

        def emit_p2(m, n, ps_pool, eng_flip):
            # deprioritize against the recurrence chain for engine contention
            tc.cur_priority += 50000
            _emit_p2_body(m, n, ps_pool, eng_flip)
            tc.cur_priority -= 50000

        def _emit_p2_body(m, n, ps_pool, eng_flip):
            nw = min(512, V - 512 * n)
            mr = min(128, BC * seq - 128 * m)
            wts = []
            for k in range(KCH):
                wt = wsb.tile([128, 512], BF, tag="wt")
                nc.sync.dma_start(
                    wt[:, 0:nw],
                    outWT_d[128 * k: 128 * k + 128, 512 * n: 512 * n + nw])
                wts.append(wt)
            ps = ps_pool.tile([128, 512], F32, tag="po")
            for k in range(KCH):
                mm(ps[0:mr, 0:nw],
                   HT[:, ht_cols * k + 4 + 128 * m:
                      ht_cols * k + 4 + 128 * m + mr],
                   wts[k][:, 0:nw],
                   start=(k == 0), stop=False)
            mm(ps[0:mr, 0:nw], ones1[0:1, 0:mr],
               outb_sb[0:1, 512 * n: 512 * n + nw],
               start=False, stop=True)
            # int8-quantize per output row: q = round(x * 127/absmax(row)),
            # scale[row] = absmax/127 shipped alongside (HW convert is
            # round-to-nearest with saturation).
            mx = qsb.tile([128, 1], F32, tag="qmx")
            nc.vector.tensor_reduce(mx[0:mr], ps[0:mr, 0:nw],
                                    axis=mybir.AxisListType.X,
                                    op=mybir.AluOpType.max,
                                    apply_absolute_value=True)
            rq = qsb.tile([128, 1], F32, tag="qrq")
            nc.vector.reciprocal(rq[0:mr], mx[0:mr])
            nc.vector.tensor_scalar_mul(rq[0:mr], rq[0:mr], 127.0)
            nc.vector.tensor_scalar_mul(
                scales[0:mr, nvoc * m + n: nvoc * m + n + 1],
                mx[0:mr], 1.0 / 127.0)
            ob = osb.tile([128, 512], I8, tag="ob")
            if eng_flip:
                nc.scalar.activation(ob[0:mr, 0:nw], ps[0:mr, 0:nw],
                                     _ACT.Copy, scale=rq[0:mr])
            else:
                nc.vector.tensor_scalar_mul(ob[0:mr, 0:nw], ps[0:mr, 0:nw],
                                            rq[0:mr])
            dst = out_d[:, 32 * m: 32 * m + mr // 4, 512 * n: 512 * n + nw]
            nc.sync.dma_start(dst.rearrange("b t v -> t b v"), ob[0:mr, 0:nw])

        # ---------------- recurrence ----------------
        with ExitStack() as rctx:
            sb = rctx.enter_context(tc.tile_pool(name="sb", bufs=2))
            ps_sc = rctx.enter_context(
                tc.tile_pool(name="ps_sc", bufs=1, space="PSUM"))
            ps_tp = rctx.enter_context(
                tc.tile_pool(name="ps_tp", bufs=1, space="PSUM"))
            ps_g = rctx.enter_context(
                tc.tile_pool(name="ps_g", bufs=1, space="PSUM"))
            ps_oi = rctx.enter_context(
                tc.tile_pool(name="ps_oi", bufs=1, space="PSUM"))
            p2_done = 0  # m=0 vocab chunks emitted inside the recurrence

            # scores psum: batch b's scores live in row 32*b (col-group
            # tile_position); untouched rows stay 0 from this one memset.
            psum_s = ps_sc.tile([128, 512], F32, tag="ps_s")
            nc.vector.memset(psum_s[:], 0.0)

            for t in range(seq):
                hc = 4 * t

                # scores row for batch b at partition 32b:
                # psum_s[32b, n] = <h_b, F[b,n,:]>
                for b in range(BC):
                    for k in range(KCH):
                        mm(psum_s[32 * b: 32 * b + 1, 0:NREG],
                           hT2[:, 8 * k + 2 * b: 8 * k + 2 * b + 1],
                           fT[:, BC * NREG * k + NREG * b:
                              BC * NREG * k + NREG * (b + 1)],
                           start=(k == 0), stop=(k == KCH - 1),
                           tile_position=(0, 32 * b))

                # gates part 1: h @ Whh.T + G_emb  (PE work hiding softmax).
                # G_emb rows for step t come from gemb_sb via an eye128
                # column-select (rows 4*(t%32)..+4 of half t//32).
                gps = ps_g.tile([BC, G], F32, tag="gps")
                tr4 = 4 * (t % 32)
                gh = G * (t // 32)
                for n in range(4):
                    gsl = slice(512 * n, 512 * n + 512)
                    for k in range(KCH):
                        mm(gps[:, gsl],
                           HT[:, ht_cols * k + hc: ht_cols * k + hc + 4],
                           whhT[:, G * k + 512 * n: G * k + 512 * n + 512],
                           start=(k == 0), stop=False)
                    mm(gps[:, gsl], eye128[:, tr4: tr4 + 4],
                       gemb_sb[:, gh + 512 * n: gh + 512 * n + 512],
                       start=False, stop=False)

                # softmax along the free dim, rows {0,32,64,96} meaningful
                mx = sb.tile([128, 1], F32, tag="mx")
                nc.vector.reduce_max(mx[:], psum_s[:, 0:NREG],
                                     axis=mybir.AxisListType.X)
                nmx = sb.tile([128, 1], F32, tag="nmx")
                nc.vector.tensor_scalar_mul(nmx[:], mx[:], -1.0)
                ssum = sb.tile([128, 1], F32, tag="ssum")
                nc.scalar.activation(attn_bf[:, 0:NREG], psum_s[:, 0:NREG], _ACT.Exp,
                                     bias=nmx[:], scale=1.0, accum_out=ssum[:])
                rinv = sb.tile([128, 1], F32, tag="rinv")
                nc.vector.reciprocal(rinv[:], ssum[:])
                nc.vector.tensor_scalar_mul(attn_bf[:, 0:NREG],
                                            attn_bf[:, 0:NREG], rinv[:])

                # attn.T via row-wise PE transposes -> block-diag scatter
                atp = ps_tp.tile([128, 4 * BC], BF, tag="tpb")
                for b in range(BC):
                    for k2 in range(2):
                        c2 = 2 * b + k2
                        mm(atp[:, 2 * c2: 2 * c2 + 1],
                           attn_bf[32 * b: 32 * b + 1,
                                   128 * k2: 128 * (k2 + 1)],
                           onescol[32 * b: 32 * b + 1, 0:1],
                           is_transpose=True, tile_position=(32 * b, 0))
                nc.vector.memset(BD[:], 0.0)
                # dst col 4*(2b+k2)+b = 9b+4k2, src col 2*(2b+k2) = 4b+2k2:
                # both affine in (b, k2) -> a single strided-AP copy
                bd_dst = bass.AP(BD.tensor, BD.offset,
                                 [BD.ap[0], [9, BC], [4, 2]])
                bd_src = bass.AP(atp.tensor, atp.offset,
                                 [atp.ap[0], [4, BC], [2, 2]])
                nc.scalar.copy(bd_dst, bd_src)

                # context transposed: ctxT[e, b]
                cps = ps_tp.tile([128, 4 * KCH], F32, tag="cps")
                for m in range(KCH):
                    for c2 in range(2 * BC):
                        mm(cps[:, 4 * m: 4 * m + 4],
                           fP[:, 512 * c2 + 128 * m: 512 * c2 + 128 * m + 128],
                           BD[:, 4 * c2: 4 * c2 + 4],
                           start=(c2 == 0), stop=(c2 == 2 * BC - 1))
                nc.scalar.copy(ctxT[:], cps[:])

                # gates part 2: ctx @ W_cg.T
                for n in range(4):
                    gsl = slice(512 * n, 512 * n + 512)
                    for k in range(KCH):
                        mm(gps[:, gsl],
                           ctxT[:, 4 * k: 4 * k + 4],
                           wcgT[:, G * k + 512 * n: G * k + 512 * n + 512],
                           start=False, stop=(k == KCH - 1))

                # vocab-projection chunks for rows t<32 interleave into the
                # idle PE window left by the elementwise chain (also keeps
                # the PE p-state warm)
                import os as _os
                if seq == SEQ and t >= 33 and not _os.environ.get("K_NO_P2"):
                    quota = min(nvoc, 2 * (t - 32))
                    while p2_done < quota:
                        emit_p2(0, p2_done, ps_oi, p2_done % 2 == 0)
                        p2_done += 1

                # LSTM cell via tanh-only activations (one ACT table set).
                # sigma(x) = 0.5(1+tanh(x/2)); h is stored as 2h with the
                # 0.5 factors folded into fT/whhT/outWT/h0T on the host, so
                # each sigma-multiply fuses into one scalar_tensor_tensor:
                #   u0 = (1+th_f)*c = 2*sig(f)*c
                #   u1 = (1+th_i)*tg = 2*sig(i)*tanh(g)
                #   v = u0+u1 = 2*c2;  c <- 0.5v;  tanh(c2) = Tanh(0.5*v)
                #   h2x2 = (1+th_o)*tanh(c2) = 2*h2
                mlop = mybir.AluOpType.mult
                adop = mybir.AluOpType.add
                thif = sb.tile([BC, 1024], F32, tag="thif")
                nc.scalar.activation(thif[:], gps[:, 0:1024], _ACT.Tanh,
                                     scale=0.5)
                tg = sb.tile([BC, 512], F32, tag="tg")
                nc.scalar.activation(tg[:], gps[:, 1024:1536], _ACT.Tanh)
                tho = sb.tile([BC, 512], F32, tag="tho")
                nc.scalar.activation(tho[:], gps[:, 1536:2048], _ACT.Tanh,
                                     scale=0.5)
                u0 = sb.tile([BC, 512], F32, tag="u0")
                nc.vector.scalar_tensor_tensor(u0[:], thif[:, 512:1024], 1.0,
                                               cst[:], adop, mlop)
                u1 = sb.tile([BC, 512], F32, tag="u1")
                nc.vector.scalar_tensor_tensor(u1[:], thif[:, 0:512], 1.0,
                                               tg[:], adop, mlop)
                v2c = sb.tile([BC, 512], F32, tag="v2c")
                nc.vector.tensor_add(v2c[:], u0[:], u1[:])
                tc2 = sb.tile([BC, 512], F32, tag="tc2")
                nc.scalar.activation(tc2[:], v2c[:], _ACT.Tanh, scale=0.5)
                nc.vector.tensor_scalar_mul(cst[:], v2c[:], 0.5)
                h2 = sb.tile([BC, 512], BF, tag="h2")
                nc.vector.scalar_tensor_tensor(h2[:], tho[:], 1.0, tc2[:],
                                               adop, mlop)

                # h2.T -> HT col group t+1
                hps = ps_tp.tile([128, 4 * KCH], BF, tag="tpb")
                for m in range(KCH):
                    nc.tensor.transpose(hps[:, 4 * m: 4 * m + 4],
                                        h2[0:4, 128 * m: 128 * m + 128],
                                        eye4[:])
                ht_dst = bass.AP(HT.tensor, HT.offset + 4 * (t + 1),
                                 [HT.ap[0], [ht_cols, KCH], [1, 4]])
                nc.scalar.copy(ht_dst, hps[:].rearrange(
                    "p (m c) -> p m c", m=KCH))
                h2_dst = bass.AP(hT2.tensor, hT2.offset,
                                 [hT2.ap[0], [8, KCH], [2, 4]])
                nc.vector.tensor_copy(h2_dst, hps[:].rearrange(
                    "p (m c) -> p m c", m=KCH))

        # ------- phase 2 tail: remaining vocab-projection chunks -----------
        import os
        if os.environ.get("K_SKIP_P2") or os.environ.get("K_NO_P2"):
            return
        with ExitStack() as ctx2:
            ps_o2 = ctx2.enter_context(
                tc.tile_pool(name="ps_o2", bufs=4, space="PSUM"))
            rest = []
            if seq == SEQ:
                try:
                    rest += [(0, n) for n in range(p2_done, nvoc)]
                except NameError:
                    rest += [(0, n) for n in range(nvoc)]
                rest += [(m, n) for m in range(1, mch) for n in range(nvoc)]
            else:
                rest += [(m, n) for m in range(mch) for n in range(nvoc)]
            for i, (m, n) in enumerate(rest):
                emit_p2(m, n, ps_o2, i % 2 == 0)
        nc.sync.dma_start(scal_d[:], scales[:])


def host_fold_weights(inputs):
    """Fold the weight tensors into the const arrays baked into the NEFF."""
    f32 = np.float32
    reshape_W = np.asarray(inputs["reshape_W"], f32)
    reshape_b = np.asarray(inputs["reshape_b"], f32)
    Wih = np.asarray(inputs["lstm_Wih"], f32)
    Whh = np.asarray(inputs["lstm_Whh"], f32)
    bih = np.asarray(inputs["lstm_bih"], f32)
    bhh = np.asarray(inputs["lstm_bhh"], f32)
    out_W = np.asarray(inputs["out_W"], f32)
    out_b = np.asarray(inputs["out_b"], f32)

    R1, R2 = reshape_W[:, :E], reshape_W[:, E:]
    W_cg = Wih @ R1
    G_W = Wih @ R2
    G_bias = reshape_b @ Wih.T + bih + bhh

    def kmajor(x):   # [512, cols] -> [128, 4*cols], col = cols*k + c
        c = x.shape[1]
        return np.ascontiguousarray(
            x.reshape(KCH, 128, c).transpose(1, 0, 2).reshape(128, KCH * c))

    return {
        "wcgT": kmajor(W_cg.T).astype(BF16),
        "whhT": kmajor(0.5 * Whh.T).astype(BF16),
        "outWT": np.ascontiguousarray(0.5 * out_W.T).astype(BF16),
        "outb": out_b.reshape(1, V).astype(BF16),
        "eye4": np.eye(BC, dtype=BF16),
        "eye128": np.eye(128, dtype=BF16),
        "gwT": kmajor(G_W.T).astype(BF16),
        "gbias": G_bias.reshape(1, G).astype(BF16),
    }


def host_prep_data(inputs, seq=SEQ):
    """Per-call data inputs -> the 8 per-core in_maps (weights excluded)."""
    f32 = np.float32
    features = np.asarray(inputs["features"], f32)
    captions = np.asarray(inputs["captions"])
    embed_W = np.asarray(inputs["embed_W"], f32)
    init_h_W = np.asarray(inputs["init_h_W"], f32)
    init_h_b = np.asarray(inputs["init_h_b"], f32)
    init_c_W = np.asarray(inputs["init_c_W"], f32)
    init_c_b = np.asarray(inputs["init_c_b"], f32)

    emb = embed_W[captions] * np.sqrt(f32(E))           # [B, S, E]
    fmean = features.mean(axis=1)
    h0 = fmean @ init_h_W.T + init_h_b
    c0 = fmean @ init_c_W.T + init_c_b

    in_maps = []
    for c in range(NCORES):
        bs = slice(BC * c, BC * (c + 1))
        Fc = features[bs]
        fP0 = Fc[:, :128].transpose(1, 0, 2).reshape(128, BC * E)
        fP1 = Fc[:, 128:NREG].transpose(1, 0, 2).reshape(NREG - 128, BC * E)
        h0T = (2.0 * h0[bs].T.reshape(KCH, 128, BC)
               .transpose(1, 0, 2).reshape(128, KCH * BC))
        # embT col = 256k + 128h + 4*t_rel + b  (t = 32h + t_rel)
        embT = (emb[bs, :seq].transpose(2, 1, 0)        # [e, t, b]
                .reshape(KCH, 128, 2, 32, BC)
                .transpose(1, 0, 2, 3, 4).reshape(128, KCH * 2 * 128))
        in_maps.append({
            "fP0": np.ascontiguousarray(fP0).astype(BF16),
            "fP1": np.ascontiguousarray(fP1).astype(BF16),
            "h0T": np.ascontiguousarray(h0T).astype(BF16),
            "c0": np.ascontiguousarray(c0[bs]),
            "embT": np.ascontiguousarray(embT).astype(BF16),
        })
    return in_maps


def assemble_output(outs):
    """Dequantize int8+scales global outputs -> [B, SEQ, V] float32.

    Output row r of row-chunk m maps to (t = 32m + r//4, b_local = r%4);
    scales column index is m*NVOC + n for vocab chunk n.
    """
    O = np.concatenate(outs["out"], axis=0)      # [B, SEQ, V] int8
    S = np.concatenate(outs["scales"], axis=0)   # [NCORES*128, MCH*NVOC] f32
    Sb = (S.reshape(NCORES, 128, MCH, NVOC)
          .reshape(NCORES, 32, BC, MCH, NVOC)
          .transpose(0, 2, 3, 1, 4)        # [core, b, m, t_rel, n]
          .reshape(B, SEQ, NVOC))
    out = np.empty((B, SEQ, V), np.float32)
    nfull = V // 512                       # full 512-wide chunks
    vf = nfull * 512
    out[:, :, :vf] = (O[:, :, :vf].reshape(B, SEQ, nfull, 512)
                      * Sb[:, :, :nfull, None]).reshape(B, SEQ, vf)
    out[:, :, vf:] = O[:, :, vf:] * Sb[:, :, nfull, None]
    return out


# ---------------------------------------------------------------------------
# Cached PJRT runner: jit the bass program ONCE per weight-set; later calls
# only pay input transfer + execute + output fetch.

class _Bundle:
    def __init__(self, nc):
        import jax
        from jax.experimental.shard_map import shard_map
        from jax.sharding import Mesh, PartitionSpec

        _bass2jax.install_neuronx_cc_hook()
        self.nc = nc
        partition_name = (nc.partition_id_tensor.name
                          if nc.partition_id_tensor else None)
        in_names, out_names, out_avals = [], [], []
        for alloc in nc.m.functions[0].allocations:
            if not isinstance(alloc, mybir.MemoryLocationSet):
                continue
            name = alloc.memorylocations[0].name
            if alloc.kind == "ExternalInput":
                if name != partition_name:
                    in_names.append(name)
            elif alloc.kind == "ExternalOutput":
                out_names.append(name)
                out_avals.append(jax.core.ShapedArray(
                    tuple(alloc.tensor_shape), mybir.dt.np(alloc.dtype)))
        self.in_names, self.out_names = in_names, out_names
        self.out_avals = out_avals
        bind_in_names = list(in_names)
        if partition_name is not None:
            bind_in_names.append(partition_name)

        def _body(*args):
            operands = list(args)
            if partition_name is not None:
                operands.append(_bass2jax.partition_id_tensor())
            outs = _bass2jax._bass_exec_p.bind(
                *operands,
                out_avals=tuple(out_avals),
                in_names=tuple(bind_in_names),
                out_names=tuple(out_names),
                lowering_input_output_aliases=(),
                sim_require_finite=True,
                sim_require_nnan=True,
                nc=nc,
            )
            return tuple(outs)

        devices = jax.devices()[:NCORES]
        assert len(devices) == NCORES
        mesh = Mesh(np.asarray(devices), ("core",))
        P = PartitionSpec
        self.jitfn = jax.jit(shard_map(
            _body, mesh=mesh,
            in_specs=(P("core"),) * len(in_names),
            out_specs=(P("core"),) * len(out_names),
            check_rep=False))

    def run(self, in_maps):
        """in_maps (host numpy, one dict per core) -> global outputs on host
        (dict name -> [NCORES*dim0, ...])."""
        import jax
        concat_in = [
            np.concatenate([m[name] for m in in_maps], axis=0)
            for name in self.in_names
        ]
        out_arrs = self.jitfn(*concat_in)
        # fetch all output shards concurrently (the axon link serializes a
        # single np.asarray shard-by-shard at ~2/3 of its aggregate rate);
        # return per-core arrays — same boundary the baseline's
        # run_bass_kernel_spmd results had — and leave the host-side
        # concatenation to assemble_output()
        shard_data, meta = [], []
        for i, arr in enumerate(out_arrs):
            d0 = self.out_avals[i].shape[0]
            for s in arr.addressable_shards:
                shard_data.append(s.data)
                meta.append((self.out_names[i], s.index[0].start // d0))
        vals = jax.device_get(shard_data)
        result = {name: [None] * NCORES for name in self.out_names}
        for (name, core), v in zip(meta, vals):
            result[name][core] = v
        return result


_bundle_cache = {}


def get_bundle(inputs):
    key = hashlib.blake2b(
        b"".join(np.ascontiguousarray(np.asarray(inputs[k], np.float32))
                 .tobytes()
                 for k in ("reshape_W", "reshape_b", "lstm_Wih", "lstm_Whh",
                           "lstm_bih", "lstm_bhh", "out_W", "out_b")),
        digest_size=16).hexdigest()
    if key not in _bundle_cache:
        consts = host_fold_weights(inputs)
        nc = build_program(consts, SEQ)
        _bundle_cache[key] = _Bundle(nc)
    return _bundle_cache[key]


def kernel(**inputs) -> np.ndarray:
    bundle = get_bundle(inputs)
    in_maps = host_prep_data(inputs, SEQ)
    outs = bundle.run(in_maps)
    return assemble_output(outs)                 # [32, SEQ, V] f32


if __name__ == "__main__":
    import reference as refmod
    inputs = {k: np.asarray(v) for k, v in refmod.setup_inputs().items()}
    expected = np.asarray(refmod.reference(**inputs))
    got = kernel(**inputs)
    err = np.abs(got - expected).max() / np.abs(expected).max()
    l2 = np.linalg.norm((got - expected).ravel()) / np.linalg.norm(expected.ravel())
    print(f"Relative error: {err:.3e} (l2 {l2:.3e})")
